# revision 9
# baseline (speedup 1.0000x reference)
"""Trainium2 Bass kernel for nn_CrossAttention (B=2, N=2048, C=1024, H=16, D=64).

Sharding: 8 cores = 2 batches x 4 head-groups (4 heads each).
Each core computes its head-group's attention + a partial output projection;
the host sums the 4 partials per batch (bf16) and adds the bias.

Device pipeline per core:
  P1: q/k/v projections (f32r matmuls), zero-mean folded into host-centered
      weights, variance via ACT Square (bf16) + DVE reduce, rstd via
      broadcast tensor_tensor, RoPE fused on DVE in bf16 (2x/4x modes),
      bf16 PE transposes into head-paired q^T/k^T tiles, gate projected in
      transposed layout (raw, sigmoid deferred).  All input tensors are
      host-side pre-permuted so every DMA is a contiguous per-partition read;
      KV-path loads go on the sync HWDGE queue, Q-path/P2 loads on the
      scalar HWDGE queue so the first matmul starts as early as possible.
  P2: per q-block: row-tiled paired score matmuls, softmax numerators via
      ACT Exp (10/16 k-chunks, exact) or a DVE exp2 bit-trick into bf16
      (6/16 k-chunks): i16 = round(score*FE_A + FE_B) bitcast to bf16,
      col-tiled paired attn@v + M=1 ones matmuls for denominators,
      tanh-based sigmoid gating fused as (tanh+1)*1/(2*dn), bf16 output
      projection, bf16 partial out written once per 128-token row block.
"""

import os
import sys
import numpy as np

for _p in ("/opt/trn_rl_repo", "/opt/pypackages"):
    if _p not in sys.path:
        sys.path.insert(0, _p)

B, N, C = 2, 2048, 1024
H, D = 16, 64
HG = 4            # heads per core
NCH = 16          # token chunks of 128
KTC = 16          # key chunks of 128
EPS = 1e-6

# fast-exp: exp(s*0.125) ~= bitcast_bf16(int16(s*FE_A + FE_B))
FE_A = float(0.125 * np.log2(np.e) * 128.0)
FE_B = float(127.0 * 128.0 - 5.0)
APPROX_KC = (2, 5, 8, 10, 13, 15)   # k-chunks computed with the DVE fast-exp

_PROG = None      # cached compiled Bass program
LAST_EXEC_NS = None
LAST_PROFILE = None


def _build_program():
    import concourse.bass as bass
    import concourse.bacc as bacc
    import concourse.tile as tile
    import concourse.mybir as mybir

    F32 = mybir.dt.float32
    F32R = mybir.dt.float32r
    BF = mybir.dt.bfloat16
    I16 = mybir.dt.int16
    AF = mybir.ActivationFunctionType
    OP = mybir.AluOpType

    nc = bacc.Bacc("TRN2", target_bir_lowering=False, debug=False, num_devices=8)

    xT = nc.dram_tensor("xT", [128, 4, 8, 512], BF, kind="ExternalInput")
    ctxT = nc.dram_tensor("ctxT", [128, 4, 8, 512], BF, kind="ExternalInput")
    wq = nc.dram_tensor("wq", [128, 8, 256], BF, kind="ExternalInput")
    wg = nc.dram_tensor("wg", [128, 8, 256], BF, kind="ExternalInput")
    wkv = nc.dram_tensor("wkv", [128, 8, 512], BF, kind="ExternalInput")
    wo = nc.dram_tensor("wo", [128, 2, 1024], BF, kind="ExternalInput")
    cosq = nc.dram_tensor("cosq", [128, NCH, D], BF, kind="ExternalInput")
    ssinq = nc.dram_tensor("ssinq", [128, NCH, D], BF, kind="ExternalInput")
    cosk = nc.dram_tensor("cosk", [128, NCH, D], BF, kind="ExternalInput")
    ssink = nc.dram_tensor("ssink", [128, NCH, D], BF, kind="ExternalInput")
    part = nc.dram_tensor("part", [N, C], BF, kind="ExternalOutput")

    def bcast4(ap):
        # [128, 64] -> [128, 4, 64] with step-0 middle dim (read-broadcast)
        return bass.AP(tensor=ap.tensor, offset=ap.offset,
                       ap=[ap.ap[0], [0, 4], ap.ap[1]])

    def bcast64(ap):
        # [128, 4] -> [128, 4, 64] with step-0 last dim (per-head scalar)
        return bass.AP(tensor=ap.tensor, offset=ap.offset,
                       ap=[ap.ap[0], ap.ap[1], [0, 64]])

    def swap_view(ap):
        # ap: [128, 4, 64] contiguous -> per head read order d+32..d+63, d..d+31
        p, hdim, ddim = ap.ap
        return bass.AP(tensor=ap.tensor, offset=ap.offset + 32 * ddim[0],
                       ap=[p, hdim, [-32 * ddim[0], 2], [ddim[0], 32]])

    with tile.TileContext(nc) as tc:
        import contextlib
        with contextlib.ExitStack() as ctx:
            singles = ctx.enter_context(tc.tile_pool(name="singles", bufs=1))
            slices = ctx.enter_context(tc.tile_pool(name="slices", bufs=2))
            work = ctx.enter_context(tc.tile_pool(name="work", bufs=3))
            persist = ctx.enter_context(tc.tile_pool(name="persist", bufs=1))
            exps_p = ctx.enter_context(tc.tile_pool(name="exps", bufs=6))
            gat_p = ctx.enter_context(tc.tile_pool(name="gat", bufs=2))

            # ---- weights / tables.  KV-path deps go first on the sync
            # queue; Q-path and P2 loads stream in parallel on the scalar
            # HWDGE queue so the first projection matmul starts early.
            wkv_sb = singles.tile([128, 8, 512], BF)
            nc.sync.dma_start(out=wkv_sb, in_=wkv.ap())
            ck_sb = singles.tile([128, NCH, D], BF)
            nc.sync.dma_start(out=ck_sb, in_=cosk.ap())
            sk_sb = singles.tile([128, NCH, D], BF)
            nc.sync.dma_start(out=sk_sb, in_=ssink.ap())

            wq_sb = singles.tile([128, 8, 256], BF)
            nc.scalar.dma_start(out=wq_sb, in_=wq.ap())
            cq_sb = singles.tile([128, NCH, D], BF)
            nc.scalar.dma_start(out=cq_sb, in_=cosq.ap())
            sq_sb = singles.tile([128, NCH, D], BF)
            nc.scalar.dma_start(out=sq_sb, in_=ssinq.ap())
            wg_sb = singles.tile([128, 8, 256], BF)
            nc.scalar.dma_start(out=wg_sb, in_=wg.ap())
            wo_sb = singles.tile([128, 2, 1024], BF)
            nc.scalar.dma_start(out=wo_sb, in_=wo.ap())

            from concourse.masks import make_identity
            ident = singles.tile([128, 128], BF)
            make_identity(nc, ident)
            ones1 = singles.tile([128, 1], BF)
            nc.vector.memset(ones1, 1.0)
            ones2 = singles.tile([128, 64], BF)
            nc.vector.memset(ones2, 1.0)
            eps_sb = singles.tile([128, 1], F32)
            nc.vector.memset(eps_sb, EPS)

            # ---- persistent intermediates ----
            pairQ = [persist.tile([128, N], BF, tag=f"pairQ{p}", name=f"pairQ{p}") for p in range(2)]
            pairK = [persist.tile([128, N], BF, tag=f"pairK{p}", name=f"pairK{p}") for p in range(2)]
            v_sb = persist.tile([128, KTC, 4, 64], BF, tag="v_sb")
            graw = persist.tile([128, 2, N], BF, tag="graw")
            A_sb = persist.tile([128, 2, N], BF, tag="A_sb")

            # ================= P1: projections / norm / rope / transposes ====
            with tc.tile_pool(name="psA", bufs=3, space="PSUM") as psA, \
                 tc.tile_pool(name="psT", bufs=4, space="PSUM") as psT:

                pend_t = []  # (qr, i, dst_pair) transposes delayed 2 chunks

                def flush_transpose(qr, i, dst_pair):
                    # PE transpose: heads (2p, 2p+1) -> pair tile slice (bf16)
                    for p in range(2):
                        pst = psT.tile([128, 128], BF, tag="tp")
                        nc.tensor.transpose(
                            pst,
                            qr[:, 2 * p:2 * p + 2, :].rearrange("p a b -> p (a b)"),
                            ident)
                        nc.scalar.activation(
                            out=dst_pair[p][:, i * 128:(i + 1) * 128],
                            in_=pst, func=AF.Copy)

                def qk_path(sl, ns, i, w_rhs, wcols, cos_t, sin_t, dst_pair):
                    """Project+norm+rope chunk i of q or k; transpose deferred."""
                    ps = psA.tile([128, 512], F32, tag="proj")
                    for c in range(8):
                        nc.tensor.matmul(ps[:, :wcols],
                                         sl[:, c, ns * 128:(ns + 1) * 128],
                                         w_rhs(c),
                                         start=(c == 0), stop=(c == 7))
                    qpart = ps[:, 0:256]
                    # variance (zero-mean folded into host-centered weights)
                    sqv = work.tile([128, 256], BF, tag="sq")
                    nc.scalar.activation(out=sqv, in_=qpart, func=AF.Square)
                    ssum = work.tile([128, 4], BF, tag="ssum")
                    with nc.allow_low_precision("rmsnorm stats tolerate bf16"):
                        nc.vector.tensor_reduce(
                            out=ssum, in_=sqv.rearrange("p (h d) -> p h d", h=4),
                            axis=mybir.AxisListType.X, op=OP.add)
                    sdev = work.tile([128, 4], F32, tag="sdev")
                    nc.scalar.activation(out=sdev, in_=ssum, func=AF.Sqrt,
                                         scale=1.0 / 64.0, bias=eps_sb)
                    rstd = work.tile([128, 4], F32, tag="rstd")
                    nc.vector.reciprocal(out=rstd, in_=sdev)
                    qs = work.tile([128, 4, 64], BF, tag="qs")
                    nc.vector.tensor_tensor(
                        out=qs, in0=qpart.rearrange("p (h d) -> p h d", h=4),
                        in1=bcast64(rstd), op=OP.mult)
                    # rope: qr = qs*cos + swap(qs)*ssin (sign folded in ssin)
                    t1 = work.tile([128, 4, 64], BF, tag="t1")
                    nc.vector.tensor_tensor(out=t1, in0=qs, in1=bcast4(cos_t),
                                            op=OP.mult)
                    t2 = work.tile([128, 4, 64], BF, tag="t2")
                    nc.vector.tensor_tensor(out=t2, in0=swap_view(qs),
                                            in1=bcast4(sin_t), op=OP.mult)
                    qr = work.tile([128, 4, 64], BF, tag="qr")
                    nc.vector.tensor_tensor(out=qr, in0=t1, in1=t2, op=OP.add)
                    pend_t.append((qr, i, dst_pair))
                    if len(pend_t) > 2:
                        flush_transpose(*pend_t.pop(0))
                    return ps

                # K/V path over all 16 chunks
                for qc in range(4):
                    c_sl = slices.tile([128, 8, 512], BF, tag="slice")
                    nc.sync.dma_start(out=c_sl, in_=ctxT.ap()[:, qc])
                    for ns in range(4):
                        j = qc * 4 + ns
                        ps = qk_path(c_sl, ns, j,
                                     lambda c: wkv_sb[:, c, :], 512,
                                     ck_sb[:, j, :], sk_sb[:, j, :], pairK)
                        nc.vector.tensor_copy(
                            out=v_sb[:, j, :, 0:64],
                            in_=ps[:, 256:512].rearrange("p (h d) -> p h d", h=4))

                # Q path + raw gate over all 16 chunks
                for qc in range(4):
                    x_sl = slices.tile([128, 8, 512], BF, tag="slice")
                    nc.sync.dma_start(out=x_sl, in_=xT.ap()[:, qc])
                    for ns in range(4):
                        i = qc * 4 + ns
                        qk_path(x_sl, ns, i,
                                lambda c: wq_sb[:, c, :], 256,
                                cq_sb[:, i, :], sq_sb[:, i, :], pairQ)
                    # gate projection, transposed layout, raw (sigmoid later)
                    for gfc in range(2):
                        psg = psA.tile([128, 512], F32, tag="proj")
                        for c in range(8):
                            nc.tensor.matmul(
                                psg, wg_sb[:, c, gfc * 128:(gfc + 1) * 128],
                                x_sl[:, c, :], start=(c == 0), stop=(c == 7))
                        nc.scalar.activation(
                            out=graw[:, gfc, qc * 512:(qc + 1) * 512], in_=psg,
                            func=AF.Copy)
                for e in pend_t:
                    flush_transpose(*e)
                del pend_t[:]

            # ================= P2: attention + gating + out-proj =============
            with tc.tile_pool(name="psSC", bufs=2, space="PSUM") as psSC, \
                 tc.tile_pool(name="psAO", bufs=2, space="PSUM") as psAO, \
                 tc.tile_pool(name="psDN", bufs=2, space="PSUM") as psDN:

                for qc in range(4):
                    qsl = slice(qc * 512, (qc + 1) * 512)
                    for p in range(2):
                        ao_p = psAO.tile([128, 512], F32, tag="ao")
                        dn_p = psDN.tile([128, 512], F32, tag="dn")
                        # tanh(graw/2) early: ACT slots it between exps, so
                        # the gating chain after the last attn MM is short
                        gs = gat_p.tile([128, 512], F32, tag="gs")
                        nc.scalar.activation(out=gs, in_=graw[:, p, qsl],
                                             func=AF.Tanh, scale=0.5)
                        pend = []  # (expS tile, ktc) awaiting attn MMs

                        def flush_attn(eS, k, ao_p=ao_p, dn_p=dn_p, p=p):
                            # noqa: closure over current pair
                            # first MM of the k==0 group clears the whole bank;
                            # the second must NOT re-clear (would drop the
                            # first's has_written bits) -> start only on MM1.
                            st = (k == 0)
                            sp = (k == KTC - 1)
                            nc.tensor.matmul(ao_p[0:64, :], v_sb[:, k, 2 * p, :],
                                             eS[:, 0, :], start=st, stop=sp,
                                             tile_position=(0, 0),
                                             skip_group_check=True)
                            nc.tensor.matmul(ao_p[64:128, :], v_sb[:, k, 2 * p + 1, :],
                                             eS[:, 1, :], start=st, stop=sp,
                                             tile_position=(0, 64),
                                             skip_group_check=True)
                            nc.tensor.matmul(dn_p[0:1, :], ones1, eS[:, 0, :],
                                             start=st, stop=sp,
                                             tile_position=(0, 0),
                                             skip_group_check=True)
                            nc.tensor.matmul(dn_p[32:33, :], ones1, eS[:, 1, :],
                                             start=st, stop=sp,
                                             tile_position=(0, 32),
                                             skip_group_check=True)

                        for k in range(KTC):
                            ksl = slice(k * 128, (k + 1) * 128)
                            ps = psSC.tile([128, 1024], F32, tag="sc")
                            nc.tensor.matmul(ps[:, 0:512],
                                             pairK[p][0:64, ksl],
                                             pairQ[p][0:64, qsl],
                                             start=True, stop=True,
                                             tile_position=(0, 0))
                            nc.tensor.matmul(ps[:, 512:1024],
                                             pairK[p][64:128, ksl],
                                             pairQ[p][64:128, qsl],
                                             start=True, stop=True,
                                             tile_position=(64, 0))
                            eS = exps_p.tile([128, 2, 512], BF, tag="expS")
                            if k in APPROX_KC:
                                nc.vector.tensor_scalar(
                                    out=eS.bitcast(I16).rearrange("p a b -> p (a b)"),
                                    in0=ps, scalar1=FE_A, scalar2=FE_B,
                                    op0=OP.mult, op1=OP.add)
                            else:
                                nc.scalar.activation(
                                    out=eS.rearrange("p a b -> p (a b)"), in_=ps,
                                    func=AF.Exp, scale=0.125)
                            pend.append((eS, k))
                            if len(pend) > 2:
                                flush_attn(*pend.pop(0))
                        for e in pend:
                            flush_attn(*e)

                        # gating for this pair: A = ao * sigmoid(g)/dn
                        #                         = ao * (tanh(g/2)+1) * 1/(2*dn)
                        dns = gat_p.tile([128, 512], BF, tag="dns")
                        nc.vector.tensor_scalar_mul(out=dns, in0=dn_p,
                                                    scalar1=2.0)
                        rbc = psDN.tile([128, 512], F32, tag="dn")
                        nc.tensor.matmul(rbc[0:64, :], ones2[0:1, :], dns[0:1, :],
                                         start=True, stop=True,
                                         tile_position=(0, 0))
                        nc.tensor.matmul(rbc[64:128, :], ones2[32:33, :],
                                         dns[32:33, :], start=True, stop=True,
                                         tile_position=(32, 64))
                        rec = gat_p.tile([128, 512], F32, tag="rec")
                        nc.vector.reciprocal(out=rec, in_=rbc)
                        m = gat_p.tile([128, 512], F32, tag="m")
                        nc.vector.scalar_tensor_tensor(
                            out=m, in0=gs, scalar=1.0, in1=rec,
                            op0=OP.add, op1=OP.mult)
                        nc.vector.tensor_tensor(out=A_sb[:, p, qsl], in0=ao_p,
                                                in1=m, op=OP.mult)

                    # output projection for this q block (bf16 partial out)
                    for nk in range(4):
                        n1 = qc * 4 + nk
                        ev = gat_p.tile([128, 1024], BF, tag="ev")
                        for oc in range(2):
                            po = psAO.tile([128, 512], F32, tag="ao")
                            for fc in range(2):
                                nc.tensor.matmul(
                                    po,
                                    A_sb[:, fc, n1 * 128:(n1 + 1) * 128],
                                    wo_sb[:, fc, oc * 512:(oc + 1) * 512],
                                    start=(fc == 0), stop=(fc == 1))
                            nc.vector.tensor_copy(
                                out=ev[:, oc * 512:(oc + 1) * 512], in_=po)
                        nc.sync.dma_start(
                            out=part.ap()[n1 * 128:(n1 + 1) * 128, :], in_=ev)

    nc.compile()
    return nc


def _prep_core(inputs, b, g, bf16):
    x = np.asarray(inputs["x"][b], dtype=np.float32)
    ctx = np.asarray(inputs["context"][b], dtype=np.float32)
    Wq = np.asarray(inputs["Wq"], dtype=np.float32).reshape(H, 2 * D, C)
    Wkv = np.asarray(inputs["Wkv"], dtype=np.float32).reshape(H, 2 * D, C)
    Wo = np.asarray(inputs["Wo"], dtype=np.float32)
    cos = np.asarray(inputs["cos"][b], dtype=np.float32)
    sin = np.asarray(inputs["sin"][b], dtype=np.float32)
    qw = np.asarray(inputs["q_norm_w"], dtype=np.float32)
    kw = np.asarray(inputs["k_norm_w"], dtype=np.float32)

    hs = slice(HG * g, HG * g + HG)
    qr = Wq[hs, :D, :]
    qr = qr - qr.mean(axis=1, keepdims=True)
    gr = Wq[hs, D:, :]
    kr = Wkv[hs, :D, :]
    kr = kr - kr.mean(axis=1, keepdims=True)
    vr = Wkv[hs, D:, :]

    sgn = np.where(np.arange(D) < D // 2, -1.0, 1.0).astype(np.float32)
    wswap = lambda w: np.concatenate([w[D // 2:], w[:D // 2]])

    def pmajor(w, cols):
        # [cols, C] weight -> transposed, partition-major [128, 8, cols]
        return np.ascontiguousarray(
            w.reshape(cols, C).T.reshape(8, 128, cols).transpose(1, 0, 2))

    def tokmajor(t):
        # [C, N] -> [128, 4, 8, 512]: partition, q-block, c-chunk, token
        return np.ascontiguousarray(
            t.reshape(8, 128, 4, 512).transpose(1, 2, 0, 3))

    def tabs(t):
        # [N, D] -> [128, 16, D] bf16
        return np.ascontiguousarray(
            t.reshape(16, 128, D).transpose(1, 0, 2)).astype(bf16)

    return {
        "xT": tokmajor(x.T).astype(bf16),
        "ctxT": tokmajor(ctx.T).astype(bf16),
        "wq": pmajor(qr, 256).astype(bf16),
        "wg": pmajor(gr, 256).astype(bf16),
        "wkv": pmajor(
            np.concatenate([kr.reshape(HG * D, C), vr.reshape(HG * D, C)], 0),
            512).astype(bf16),
        "wo": np.ascontiguousarray(
            Wo[:, 256 * g:256 * (g + 1)].T.reshape(2, 128, C)
            .transpose(1, 0, 2)).astype(bf16),
        "cosq": tabs(cos * qw[None, :]),
        "ssinq": tabs(sin * sgn[None, :] * wswap(qw)[None, :]),
        "cosk": tabs(cos * kw[None, :]),
        "ssink": tabs(sin * sgn[None, :] * wswap(kw)[None, :]),
    }


def kernel(**inputs):
    global _PROG, LAST_EXEC_NS, LAST_PROFILE
    import ml_dtypes
    bf16 = ml_dtypes.bfloat16

    if _PROG is None:
        _PROG = _build_program()
    nc = _PROG

    in_maps = [_prep_core(inputs, core // 4, core % 4, bf16) for core in range(8)]

    trace = bool(os.environ.get("BASS_KERNEL_TRACE"))
    kw = {}
    if trace:
        import types
        from trn_agent_boot.trn_boot import _ntff_profile_via_ctypes
        hook = _ntff_profile_via_ctypes('/opt/axon/libaxon_pjrt.so')
        mod = types.ModuleType('antenv.axon_hooks')
        mod.get_axon_ntff_profile_hook = lambda: hook
        sys.modules['antenv.axon_hooks'] = mod
        from concourse import bass_utils
        bass_utils.upload_artifacts = lambda tmpdir: tmpdir
        kw = dict(trace=True, tmpdir=os.environ.get("BASS_KERNEL_TRACE_DIR"))

    from concourse.bass_utils import run_bass_kernel_spmd
    res = run_bass_kernel_spmd(nc, in_maps, core_ids=list(range(8)), **kw)
    LAST_EXEC_NS = res.exec_time_ns
    LAST_PROFILE = res.profile_json

    bo = np.asarray(inputs["bo"], dtype=np.float32)
    out = np.zeros((B, N, C), dtype=np.float32)
    for core in range(8):
        out[core // 4] += np.asarray(res.results[core]["part"],
                                     dtype=np.float32)
    out += bo[None, None, :]
    return out


# revision 10
# speedup vs baseline: 1.0109x; 1.0109x over previous
"""Trainium2 Bass kernel for nn_CrossAttention (B=2, N=2048, C=1024, H=16, D=64).

Sharding: 8 cores = 2 batches x 4 head-groups (4 heads each).
Each core computes its head-group's attention + a partial output projection;
the host sums the 4 partials per batch (bf16) and adds the bias.

Device pipeline per core:
  P1: q/k/v projections (f32r matmuls), zero-mean folded into host-centered
      weights, variance via ACT Square (bf16) + DVE reduce, rstd via
      broadcast tensor_tensor, RoPE fused on DVE in bf16 (2x/4x modes),
      bf16 PE transposes into head-paired q^T/k^T tiles, gate projected in
      transposed layout (raw, sigmoid deferred).  All input tensors are
      host-side pre-permuted so every DMA is a contiguous per-partition read;
      KV-path loads go on the sync HWDGE queue, Q-path/P2 loads on the
      scalar HWDGE queue so the first matmul starts as early as possible.
  P2: per q-block: row-tiled paired score matmuls, softmax numerators via
      ACT Exp (10/16 k-chunks, exact) or a DVE exp2 bit-trick into bf16
      (6/16 k-chunks): i16 = round(score*FE_A + FE_B) bitcast to bf16,
      col-tiled paired attn@v + M=1 ones matmuls for denominators,
      tanh-based sigmoid gating fused as (tanh+1)*1/(2*dn), bf16 output
      projection, bf16 partial out written once per 128-token row block.
"""

import os
import sys
import numpy as np

for _p in ("/opt/trn_rl_repo", "/opt/pypackages"):
    if _p not in sys.path:
        sys.path.insert(0, _p)

B, N, C = 2, 2048, 1024
H, D = 16, 64
HG = 4            # heads per core
NCH = 16          # token chunks of 128
KTC = 16          # key chunks of 128
EPS = 1e-6

# fast-exp: exp(s*0.125) ~= bitcast_bf16(int16(s*FE_A + FE_B))
FE_A = float(0.125 * np.log2(np.e) * 128.0)
FE_B = float(127.0 * 128.0 - 5.0)
APPROX_KC = (2, 5, 8, 10, 13, 15)   # k-chunks computed with the DVE fast-exp

_PROG = None      # cached compiled Bass program
LAST_EXEC_NS = None
LAST_PROFILE = None


def _build_program():
    import concourse.bass as bass
    import concourse.bacc as bacc
    import concourse.tile as tile
    import concourse.mybir as mybir

    F32 = mybir.dt.float32
    F32R = mybir.dt.float32r
    BF = mybir.dt.bfloat16
    I16 = mybir.dt.int16
    AF = mybir.ActivationFunctionType
    OP = mybir.AluOpType

    nc = bacc.Bacc("TRN2", target_bir_lowering=False, debug=False, num_devices=8)

    xT = nc.dram_tensor("xT", [128, 4, 8, 512], BF, kind="ExternalInput")
    ctxT = nc.dram_tensor("ctxT", [128, 4, 8, 512], BF, kind="ExternalInput")
    wq = nc.dram_tensor("wq", [128, 8, 256], BF, kind="ExternalInput")
    wg = nc.dram_tensor("wg", [128, 8, 256], BF, kind="ExternalInput")
    wkv = nc.dram_tensor("wkv", [128, 8, 512], BF, kind="ExternalInput")
    wo = nc.dram_tensor("wo", [128, 2, 1024], BF, kind="ExternalInput")
    cosq = nc.dram_tensor("cosq", [128, NCH, D], BF, kind="ExternalInput")
    ssinq = nc.dram_tensor("ssinq", [128, NCH, D], BF, kind="ExternalInput")
    cosk = nc.dram_tensor("cosk", [128, NCH, D], BF, kind="ExternalInput")
    ssink = nc.dram_tensor("ssink", [128, NCH, D], BF, kind="ExternalInput")
    part = nc.dram_tensor("part", [N, C], BF, kind="ExternalOutput")

    def bcast4(ap):
        # [128, 64] -> [128, 4, 64] with step-0 middle dim (read-broadcast)
        return bass.AP(tensor=ap.tensor, offset=ap.offset,
                       ap=[ap.ap[0], [0, 4], ap.ap[1]])

    def bcast64(ap):
        # [128, 4] -> [128, 4, 64] with step-0 last dim (per-head scalar)
        return bass.AP(tensor=ap.tensor, offset=ap.offset,
                       ap=[ap.ap[0], ap.ap[1], [0, 64]])

    def swap_view(ap):
        # ap: [128, 4, 64] contiguous -> per head read order d+32..d+63, d..d+31
        p, hdim, ddim = ap.ap
        return bass.AP(tensor=ap.tensor, offset=ap.offset + 32 * ddim[0],
                       ap=[p, hdim, [-32 * ddim[0], 2], [ddim[0], 32]])

    with tile.TileContext(nc) as tc:
        import contextlib
        with contextlib.ExitStack() as ctx:
            singles = ctx.enter_context(tc.tile_pool(name="singles", bufs=1))
            slices = ctx.enter_context(tc.tile_pool(name="slices", bufs=2))
            work = ctx.enter_context(tc.tile_pool(name="work", bufs=3))
            persist = ctx.enter_context(tc.tile_pool(name="persist", bufs=1))
            exps_p = ctx.enter_context(tc.tile_pool(name="exps", bufs=6))
            gat_p = ctx.enter_context(tc.tile_pool(name="gat", bufs=2))

            # ---- weights / tables.  KV-path deps go first on the sync
            # queue; Q-path and P2 loads stream in parallel on the scalar
            # HWDGE queue so the first projection matmul starts early.
            wkv_sb = singles.tile([128, 8, 512], BF)
            c_sl0 = singles.tile([128, 8, 512], BF)
            for c in range(8):
                nc.sync.dma_start(out=wkv_sb[:, c], in_=wkv.ap()[:, c])
                nc.sync.dma_start(out=c_sl0[:, c], in_=ctxT.ap()[:, 0, c])
            ck_sb = singles.tile([128, NCH, D], BF)
            nc.sync.dma_start(out=ck_sb, in_=cosk.ap())
            sk_sb = singles.tile([128, NCH, D], BF)
            nc.sync.dma_start(out=sk_sb, in_=ssink.ap())

            wq_sb = singles.tile([128, 8, 256], BF)
            nc.scalar.dma_start(out=wq_sb, in_=wq.ap())
            cq_sb = singles.tile([128, NCH, D], BF)
            nc.scalar.dma_start(out=cq_sb, in_=cosq.ap())
            sq_sb = singles.tile([128, NCH, D], BF)
            nc.scalar.dma_start(out=sq_sb, in_=ssinq.ap())
            wg_sb = singles.tile([128, 8, 256], BF)
            nc.scalar.dma_start(out=wg_sb, in_=wg.ap())
            wo_sb = singles.tile([128, 2, 1024], BF)
            nc.scalar.dma_start(out=wo_sb, in_=wo.ap())

            from concourse.masks import make_identity
            ident = singles.tile([128, 128], BF)
            make_identity(nc, ident)
            ones1 = singles.tile([128, 1], BF)
            nc.vector.memset(ones1, 1.0)
            ones2 = singles.tile([128, 64], BF)
            nc.vector.memset(ones2, 1.0)
            eps_sb = singles.tile([128, 1], F32)
            nc.vector.memset(eps_sb, EPS)

            # ---- persistent intermediates ----
            pairQ = [persist.tile([128, N], BF, tag=f"pairQ{p}", name=f"pairQ{p}") for p in range(2)]
            pairK = [persist.tile([128, N], BF, tag=f"pairK{p}", name=f"pairK{p}") for p in range(2)]
            v_sb = persist.tile([128, KTC, 4, 64], BF, tag="v_sb")
            graw = persist.tile([128, 2, N], BF, tag="graw")
            A_sb = persist.tile([128, 2, N], BF, tag="A_sb")

            # ================= P1: projections / norm / rope / transposes ====
            with tc.tile_pool(name="psA", bufs=3, space="PSUM") as psA, \
                 tc.tile_pool(name="psT", bufs=4, space="PSUM") as psT:

                pend_t = []  # (qr, i, dst_pair) transposes delayed 2 chunks

                def flush_transpose(qr, i, dst_pair):
                    # PE transpose: heads (2p, 2p+1) -> pair tile slice (bf16)
                    for p in range(2):
                        pst = psT.tile([128, 128], BF, tag="tp")
                        nc.tensor.transpose(
                            pst,
                            qr[:, 2 * p:2 * p + 2, :].rearrange("p a b -> p (a b)"),
                            ident)
                        nc.scalar.activation(
                            out=dst_pair[p][:, i * 128:(i + 1) * 128],
                            in_=pst, func=AF.Copy)

                def qk_path(sl, ns, i, w_rhs, wcols, cos_t, sin_t, dst_pair):
                    """Project+norm+rope chunk i of q or k; transpose deferred."""
                    ps = psA.tile([128, 512], F32, tag="proj")
                    for c in range(8):
                        nc.tensor.matmul(ps[:, :wcols],
                                         sl[:, c, ns * 128:(ns + 1) * 128],
                                         w_rhs(c),
                                         start=(c == 0), stop=(c == 7))
                    qpart = ps[:, 0:256]
                    # variance (zero-mean folded into host-centered weights)
                    sqv = work.tile([128, 256], BF, tag="sq")
                    nc.scalar.activation(out=sqv, in_=qpart, func=AF.Square)
                    ssum = work.tile([128, 4], BF, tag="ssum")
                    with nc.allow_low_precision("rmsnorm stats tolerate bf16"):
                        nc.vector.tensor_reduce(
                            out=ssum, in_=sqv.rearrange("p (h d) -> p h d", h=4),
                            axis=mybir.AxisListType.X, op=OP.add)
                    sdev = work.tile([128, 4], F32, tag="sdev")
                    nc.scalar.activation(out=sdev, in_=ssum, func=AF.Sqrt,
                                         scale=1.0 / 64.0, bias=eps_sb)
                    rstd = work.tile([128, 4], F32, tag="rstd")
                    nc.vector.reciprocal(out=rstd, in_=sdev)
                    qs = work.tile([128, 4, 64], BF, tag="qs")
                    nc.vector.tensor_tensor(
                        out=qs, in0=qpart.rearrange("p (h d) -> p h d", h=4),
                        in1=bcast64(rstd), op=OP.mult)
                    # rope: qr = qs*cos + swap(qs)*ssin (sign folded in ssin)
                    t1 = work.tile([128, 4, 64], BF, tag="t1")
                    nc.vector.tensor_tensor(out=t1, in0=qs, in1=bcast4(cos_t),
                                            op=OP.mult)
                    t2 = work.tile([128, 4, 64], BF, tag="t2")
                    nc.vector.tensor_tensor(out=t2, in0=swap_view(qs),
                                            in1=bcast4(sin_t), op=OP.mult)
                    qr = work.tile([128, 4, 64], BF, tag="qr")
                    nc.vector.tensor_tensor(out=qr, in0=t1, in1=t2, op=OP.add)
                    pend_t.append((qr, i, dst_pair))
                    if len(pend_t) > 2:
                        flush_transpose(*pend_t.pop(0))
                    return ps

                # K/V path over all 16 chunks (qc=0 slice preloaded above)
                for qc in range(4):
                    if qc == 0:
                        c_sl = c_sl0
                    else:
                        c_sl = slices.tile([128, 8, 512], BF, tag="slice")
                        nc.sync.dma_start(out=c_sl, in_=ctxT.ap()[:, qc])
                    for ns in range(4):
                        j = qc * 4 + ns
                        ps = qk_path(c_sl, ns, j,
                                     lambda c: wkv_sb[:, c, :], 512,
                                     ck_sb[:, j, :], sk_sb[:, j, :], pairK)
                        nc.vector.tensor_copy(
                            out=v_sb[:, j, :, 0:64],
                            in_=ps[:, 256:512].rearrange("p (h d) -> p h d", h=4))

                # Q path + raw gate over all 16 chunks
                for qc in range(4):
                    x_sl = slices.tile([128, 8, 512], BF, tag="slice")
                    nc.sync.dma_start(out=x_sl, in_=xT.ap()[:, qc])
                    for ns in range(4):
                        i = qc * 4 + ns
                        qk_path(x_sl, ns, i,
                                lambda c: wq_sb[:, c, :], 256,
                                cq_sb[:, i, :], sq_sb[:, i, :], pairQ)
                    # gate projection, transposed layout, raw (sigmoid later)
                    for gfc in range(2):
                        psg = psA.tile([128, 512], F32, tag="proj")
                        for c in range(8):
                            nc.tensor.matmul(
                                psg, wg_sb[:, c, gfc * 128:(gfc + 1) * 128],
                                x_sl[:, c, :], start=(c == 0), stop=(c == 7))
                        nc.scalar.activation(
                            out=graw[:, gfc, qc * 512:(qc + 1) * 512], in_=psg,
                            func=AF.Copy)
                for e in pend_t:
                    flush_transpose(*e)
                del pend_t[:]

            # ================= P2: attention + gating + out-proj =============
            with tc.tile_pool(name="psSC", bufs=2, space="PSUM") as psSC, \
                 tc.tile_pool(name="psAO", bufs=2, space="PSUM") as psAO, \
                 tc.tile_pool(name="psDN", bufs=2, space="PSUM") as psDN:

                for qc in range(4):
                    qsl = slice(qc * 512, (qc + 1) * 512)
                    gat = []  # (ao_p, gs, dns) per pair, gating deferred
                    for p in range(2):
                        ao_p = psAO.tile([128, 512], F32, tag="ao")
                        dn_p = psDN.tile([128, 512], F32, tag="dn")
                        # tanh(graw/2) early: ACT slots it between exps, so
                        # the gating chain after the last attn MM is short
                        gs = gat_p.tile([128, 512], F32, tag="gs")
                        nc.scalar.activation(out=gs, in_=graw[:, p, qsl],
                                             func=AF.Tanh, scale=0.5)
                        pend = []  # (expS tile, ktc) awaiting attn MMs

                        def flush_attn(eS, k, ao_p=ao_p, dn_p=dn_p, p=p):
                            # noqa: closure over current pair
                            # first MM of the k==0 group clears the whole bank;
                            # the second must NOT re-clear (would drop the
                            # first's has_written bits) -> start only on MM1.
                            st = (k == 0)
                            sp = (k == KTC - 1)
                            nc.tensor.matmul(ao_p[0:64, :], v_sb[:, k, 2 * p, :],
                                             eS[:, 0, :], start=st, stop=sp,
                                             tile_position=(0, 0),
                                             skip_group_check=True)
                            nc.tensor.matmul(ao_p[64:128, :], v_sb[:, k, 2 * p + 1, :],
                                             eS[:, 1, :], start=st, stop=sp,
                                             tile_position=(0, 64),
                                             skip_group_check=True)
                            nc.tensor.matmul(dn_p[0:1, :], ones1, eS[:, 0, :],
                                             start=st, stop=sp,
                                             tile_position=(0, 0),
                                             skip_group_check=True)
                            nc.tensor.matmul(dn_p[32:33, :], ones1, eS[:, 1, :],
                                             start=st, stop=sp,
                                             tile_position=(0, 32),
                                             skip_group_check=True)

                        for k in range(KTC):
                            ksl = slice(k * 128, (k + 1) * 128)
                            ps = psSC.tile([128, 1024], F32, tag="sc")
                            nc.tensor.matmul(ps[:, 0:512],
                                             pairK[p][0:64, ksl],
                                             pairQ[p][0:64, qsl],
                                             start=True, stop=True,
                                             tile_position=(0, 0))
                            nc.tensor.matmul(ps[:, 512:1024],
                                             pairK[p][64:128, ksl],
                                             pairQ[p][64:128, qsl],
                                             start=True, stop=True,
                                             tile_position=(64, 0))
                            eS = exps_p.tile([128, 2, 512], BF, tag="expS")
                            if k in APPROX_KC:
                                nc.vector.tensor_scalar(
                                    out=eS.bitcast(I16).rearrange("p a b -> p (a b)"),
                                    in0=ps, scalar1=FE_A, scalar2=FE_B,
                                    op0=OP.mult, op1=OP.add)
                            else:
                                nc.scalar.activation(
                                    out=eS.rearrange("p a b -> p (a b)"), in_=ps,
                                    func=AF.Exp, scale=0.125)
                            pend.append((eS, k))
                            if len(pend) > 2:
                                flush_attn(*pend.pop(0))
                        for e in pend:
                            flush_attn(*e)

                        # dns right away (DVE only, does not block the PE);
                        # the PE part of the gating chain is deferred so it
                        # runs while the other pair's matmuls keep PE busy.
                        dns = gat_p.tile([128, 512], BF, tag="dns")
                        nc.vector.tensor_scalar_mul(out=dns, in0=dn_p,
                                                    scalar1=2.0)
                        gat.append((ao_p, gs, dns))

                    # gating: A = ao * sigmoid(g)/dn = ao*(tanh(g/2)+1)/(2*dn)
                    for p, (ao_p, gs, dns) in enumerate(gat):
                        rbc = psDN.tile([128, 512], F32, tag="dn")
                        nc.tensor.matmul(rbc[0:64, :], ones2[0:1, :], dns[0:1, :],
                                         start=True, stop=True,
                                         tile_position=(0, 0))
                        nc.tensor.matmul(rbc[64:128, :], ones2[32:33, :],
                                         dns[32:33, :], start=True, stop=True,
                                         tile_position=(32, 64))
                        rec = gat_p.tile([128, 512], F32, tag="rec")
                        nc.vector.reciprocal(out=rec, in_=rbc)
                        m = gat_p.tile([128, 512], F32, tag="m")
                        nc.vector.scalar_tensor_tensor(
                            out=m, in0=gs, scalar=1.0, in1=rec,
                            op0=OP.add, op1=OP.mult)
                        nc.vector.tensor_tensor(out=A_sb[:, p, qsl], in0=ao_p,
                                                in1=m, op=OP.mult)

                    # output projection for this q block (bf16 partial out)
                    for nk in range(4):
                        n1 = qc * 4 + nk
                        ev = gat_p.tile([128, 1024], BF, tag="ev")
                        for oc in range(2):
                            po = psAO.tile([128, 512], F32, tag="ao")
                            for fc in range(2):
                                nc.tensor.matmul(
                                    po,
                                    A_sb[:, fc, n1 * 128:(n1 + 1) * 128],
                                    wo_sb[:, fc, oc * 512:(oc + 1) * 512],
                                    start=(fc == 0), stop=(fc == 1))
                            nc.vector.tensor_copy(
                                out=ev[:, oc * 512:(oc + 1) * 512], in_=po)
                        nc.sync.dma_start(
                            out=part.ap()[n1 * 128:(n1 + 1) * 128, :], in_=ev)

    nc.compile()
    return nc


def _prep_core(inputs, b, g, bf16):
    x = np.asarray(inputs["x"][b], dtype=np.float32)
    ctx = np.asarray(inputs["context"][b], dtype=np.float32)
    Wq = np.asarray(inputs["Wq"], dtype=np.float32).reshape(H, 2 * D, C)
    Wkv = np.asarray(inputs["Wkv"], dtype=np.float32).reshape(H, 2 * D, C)
    Wo = np.asarray(inputs["Wo"], dtype=np.float32)
    cos = np.asarray(inputs["cos"][b], dtype=np.float32)
    sin = np.asarray(inputs["sin"][b], dtype=np.float32)
    qw = np.asarray(inputs["q_norm_w"], dtype=np.float32)
    kw = np.asarray(inputs["k_norm_w"], dtype=np.float32)

    hs = slice(HG * g, HG * g + HG)
    qr = Wq[hs, :D, :]
    qr = qr - qr.mean(axis=1, keepdims=True)
    gr = Wq[hs, D:, :]
    kr = Wkv[hs, :D, :]
    kr = kr - kr.mean(axis=1, keepdims=True)
    vr = Wkv[hs, D:, :]

    sgn = np.where(np.arange(D) < D // 2, -1.0, 1.0).astype(np.float32)
    wswap = lambda w: np.concatenate([w[D // 2:], w[:D // 2]])

    def pmajor(w, cols):
        # [cols, C] weight -> transposed, partition-major [128, 8, cols]
        return np.ascontiguousarray(
            w.reshape(cols, C).T.reshape(8, 128, cols).transpose(1, 0, 2))

    def tokmajor(t):
        # [C, N] -> [128, 4, 8, 512]: partition, q-block, c-chunk, token
        return np.ascontiguousarray(
            t.reshape(8, 128, 4, 512).transpose(1, 2, 0, 3))

    def tabs(t):
        # [N, D] -> [128, 16, D] bf16
        return np.ascontiguousarray(
            t.reshape(16, 128, D).transpose(1, 0, 2)).astype(bf16)

    return {
        "xT": tokmajor(x.T).astype(bf16),
        "ctxT": tokmajor(ctx.T).astype(bf16),
        "wq": pmajor(qr, 256).astype(bf16),
        "wg": pmajor(gr, 256).astype(bf16),
        "wkv": pmajor(
            np.concatenate([kr.reshape(HG * D, C), vr.reshape(HG * D, C)], 0),
            512).astype(bf16),
        "wo": np.ascontiguousarray(
            Wo[:, 256 * g:256 * (g + 1)].T.reshape(2, 128, C)
            .transpose(1, 0, 2)).astype(bf16),
        "cosq": tabs(cos * qw[None, :]),
        "ssinq": tabs(sin * sgn[None, :] * wswap(qw)[None, :]),
        "cosk": tabs(cos * kw[None, :]),
        "ssink": tabs(sin * sgn[None, :] * wswap(kw)[None, :]),
    }


def kernel(**inputs):
    global _PROG, LAST_EXEC_NS, LAST_PROFILE
    import ml_dtypes
    bf16 = ml_dtypes.bfloat16

    if _PROG is None:
        _PROG = _build_program()
    nc = _PROG

    in_maps = [_prep_core(inputs, core // 4, core % 4, bf16) for core in range(8)]

    trace = bool(os.environ.get("BASS_KERNEL_TRACE"))
    kw = {}
    if trace:
        import types
        from trn_agent_boot.trn_boot import _ntff_profile_via_ctypes
        hook = _ntff_profile_via_ctypes('/opt/axon/libaxon_pjrt.so')
        mod = types.ModuleType('antenv.axon_hooks')
        mod.get_axon_ntff_profile_hook = lambda: hook
        sys.modules['antenv.axon_hooks'] = mod
        from concourse import bass_utils
        bass_utils.upload_artifacts = lambda tmpdir: tmpdir
        kw = dict(trace=True, tmpdir=os.environ.get("BASS_KERNEL_TRACE_DIR"))

    from concourse.bass_utils import run_bass_kernel_spmd
    res = run_bass_kernel_spmd(nc, in_maps, core_ids=list(range(8)), **kw)
    LAST_EXEC_NS = res.exec_time_ns
    LAST_PROFILE = res.profile_json

    bo = np.asarray(inputs["bo"], dtype=np.float32)
    out = np.zeros((B, N, C), dtype=np.float32)
    for core in range(8):
        out[core // 4] += np.asarray(res.results[core]["part"],
                                     dtype=np.float32)
    out += bo[None, None, :]
    return out


# revision 11
# speedup vs baseline: 1.0157x; 1.0048x over previous
"""Trainium2 Bass kernel for nn_CrossAttention (B=2, N=2048, C=1024, H=16, D=64).

Sharding: 8 cores = 2 batches x 4 head-groups (4 heads each).
Each core computes its head-group's attention + a partial output projection;
the host sums the 4 partials per batch (bf16) and adds the bias.

Device pipeline per core:
  P1: q/k/v projections (f32r matmuls), zero-mean folded into host-centered
      weights, variance via ACT Square (bf16) + DVE reduce, rstd via
      broadcast tensor_tensor, RoPE fused on DVE in bf16 (2x/4x modes),
      bf16 PE transposes into head-paired q^T/k^T tiles, gate projected in
      transposed layout (raw, sigmoid deferred).  All input tensors are
      host-side pre-permuted so every DMA is a contiguous per-partition read;
      KV-path loads go on the sync HWDGE queue, Q-path/P2 loads on the
      scalar HWDGE queue so the first matmul starts as early as possible.
  P2: per q-block: row-tiled paired score matmuls, softmax numerators via
      ACT Exp (10/16 k-chunks, exact) or a DVE exp2 bit-trick into bf16
      (6/16 k-chunks): i16 = round(score*FE_A + FE_B) bitcast to bf16,
      col-tiled paired attn@v + M=1 ones matmuls for denominators,
      tanh-based sigmoid gating fused as (tanh+1)*1/(2*dn), bf16 output
      projection, bf16 partial out written once per 128-token row block.
"""

import os
import sys
import numpy as np

for _p in ("/opt/trn_rl_repo", "/opt/pypackages"):
    if _p not in sys.path:
        sys.path.insert(0, _p)

B, N, C = 2, 2048, 1024
H, D = 16, 64
HG = 4            # heads per core
NCH = 16          # token chunks of 128
KTC = 16          # key chunks of 128
EPS = 1e-6

# fast-exp: exp(s*0.125) ~= bitcast_bf16(int16(s*FE_A + FE_B))
FE_A = float(0.125 * np.log2(np.e) * 128.0)
FE_B = float(127.0 * 128.0 - 5.0)
APPROX_KC = (1, 3, 5, 7, 9, 11)   # DVE fast-exp chunks, early in the pair

_PROG = None      # cached compiled Bass program
LAST_EXEC_NS = None
LAST_PROFILE = None


def _build_program():
    import concourse.bass as bass
    import concourse.bacc as bacc
    import concourse.tile as tile
    import concourse.mybir as mybir

    F32 = mybir.dt.float32
    F32R = mybir.dt.float32r
    BF = mybir.dt.bfloat16
    I16 = mybir.dt.int16
    AF = mybir.ActivationFunctionType
    OP = mybir.AluOpType

    nc = bacc.Bacc("TRN2", target_bir_lowering=False, debug=False, num_devices=8)

    xT = nc.dram_tensor("xT", [128, 4, 8, 512], BF, kind="ExternalInput")
    ctxT = nc.dram_tensor("ctxT", [128, 4, 8, 512], BF, kind="ExternalInput")
    wq = nc.dram_tensor("wq", [128, 8, 256], BF, kind="ExternalInput")
    wg = nc.dram_tensor("wg", [128, 8, 256], BF, kind="ExternalInput")
    wkv = nc.dram_tensor("wkv", [128, 8, 512], BF, kind="ExternalInput")
    wo = nc.dram_tensor("wo", [128, 2, 1024], BF, kind="ExternalInput")
    cosq = nc.dram_tensor("cosq", [128, NCH, D], BF, kind="ExternalInput")
    ssinq = nc.dram_tensor("ssinq", [128, NCH, D], BF, kind="ExternalInput")
    cosk = nc.dram_tensor("cosk", [128, NCH, D], BF, kind="ExternalInput")
    ssink = nc.dram_tensor("ssink", [128, NCH, D], BF, kind="ExternalInput")
    part = nc.dram_tensor("part", [N, C], BF, kind="ExternalOutput")

    def bcast4(ap):
        # [128, 64] -> [128, 4, 64] with step-0 middle dim (read-broadcast)
        return bass.AP(tensor=ap.tensor, offset=ap.offset,
                       ap=[ap.ap[0], [0, 4], ap.ap[1]])

    def bcast64(ap):
        # [128, 4] -> [128, 4, 64] with step-0 last dim (per-head scalar)
        return bass.AP(tensor=ap.tensor, offset=ap.offset,
                       ap=[ap.ap[0], ap.ap[1], [0, 64]])

    def swap_view(ap):
        # ap: [128, 4, 64] contiguous -> per head read order d+32..d+63, d..d+31
        p, hdim, ddim = ap.ap
        return bass.AP(tensor=ap.tensor, offset=ap.offset + 32 * ddim[0],
                       ap=[p, hdim, [-32 * ddim[0], 2], [ddim[0], 32]])

    with tile.TileContext(nc) as tc:
        import contextlib
        with contextlib.ExitStack() as ctx:
            singles = ctx.enter_context(tc.tile_pool(name="singles", bufs=1))
            slices = ctx.enter_context(tc.tile_pool(name="slices", bufs=2))
            work = ctx.enter_context(tc.tile_pool(name="work", bufs=3))
            persist = ctx.enter_context(tc.tile_pool(name="persist", bufs=1))
            exps_p = ctx.enter_context(tc.tile_pool(name="exps", bufs=6))
            gat_p = ctx.enter_context(tc.tile_pool(name="gat", bufs=2))

            # ---- weights / tables.  KV-path deps go first on the sync
            # queue; Q-path and P2 loads stream in parallel on the scalar
            # HWDGE queue so the first projection matmul starts early.
            ck_sb = singles.tile([128, NCH, D], BF)
            nc.sync.dma_start(out=ck_sb, in_=cosk.ap())
            sk_sb = singles.tile([128, NCH, D], BF)
            nc.sync.dma_start(out=sk_sb, in_=ssink.ap())
            wkv_sb = singles.tile([128, 8, 512], BF)
            c_sl0 = singles.tile([128, 8, 512], BF)
            for c in range(8):
                nc.sync.dma_start(out=wkv_sb[:, c], in_=wkv.ap()[:, c])
                nc.sync.dma_start(out=c_sl0[:, c], in_=ctxT.ap()[:, 0, c])

            wq_sb = singles.tile([128, 8, 256], BF)
            nc.scalar.dma_start(out=wq_sb, in_=wq.ap())
            cq_sb = singles.tile([128, NCH, D], BF)
            nc.scalar.dma_start(out=cq_sb, in_=cosq.ap())
            sq_sb = singles.tile([128, NCH, D], BF)
            nc.scalar.dma_start(out=sq_sb, in_=ssinq.ap())
            wg_sb = singles.tile([128, 8, 256], BF)
            nc.scalar.dma_start(out=wg_sb, in_=wg.ap())
            wo_sb = singles.tile([128, 2, 1024], BF)
            nc.scalar.dma_start(out=wo_sb, in_=wo.ap())

            from concourse.masks import make_identity
            ident = singles.tile([128, 128], BF)
            make_identity(nc, ident)
            ones1 = singles.tile([128, 1], BF)
            nc.vector.memset(ones1, 1.0)
            ones2 = singles.tile([128, 64], BF)
            nc.vector.memset(ones2, 1.0)
            eps_sb = singles.tile([128, 1], F32)
            nc.vector.memset(eps_sb, EPS)

            # ---- persistent intermediates ----
            pairQ = [persist.tile([128, N], BF, tag=f"pairQ{p}", name=f"pairQ{p}") for p in range(2)]
            pairK = [persist.tile([128, N], BF, tag=f"pairK{p}", name=f"pairK{p}") for p in range(2)]
            v_sb = persist.tile([128, KTC, 4, 64], BF, tag="v_sb")
            graw = persist.tile([128, 2, N], BF, tag="graw")
            A_sb = persist.tile([128, 2, N], BF, tag="A_sb")

            # ================= P1: projections / norm / rope / transposes ====
            with tc.tile_pool(name="psA", bufs=3, space="PSUM") as psA, \
                 tc.tile_pool(name="psT", bufs=4, space="PSUM") as psT:

                pend_t = []  # (qr, i, dst_pair) transposes delayed 2 chunks

                def flush_transpose(qr, i, dst_pair):
                    # PE transpose: heads (2p, 2p+1) -> pair tile slice (bf16)
                    for p in range(2):
                        pst = psT.tile([128, 128], BF, tag="tp")
                        nc.tensor.transpose(
                            pst,
                            qr[:, 2 * p:2 * p + 2, :].rearrange("p a b -> p (a b)"),
                            ident)
                        nc.scalar.activation(
                            out=dst_pair[p][:, i * 128:(i + 1) * 128],
                            in_=pst, func=AF.Copy)

                def qk_path(sl, ns, i, w_rhs, wcols, cos_t, sin_t, dst_pair):
                    """Project+norm+rope chunk i of q or k; transpose deferred."""
                    ps = psA.tile([128, 512], F32, tag="proj")
                    for c in range(8):
                        nc.tensor.matmul(ps[:, :wcols],
                                         sl[:, c, ns * 128:(ns + 1) * 128],
                                         w_rhs(c),
                                         start=(c == 0), stop=(c == 7))
                    qpart = ps[:, 0:256]
                    # variance (zero-mean folded into host-centered weights)
                    sqv = work.tile([128, 256], BF, tag="sq")
                    nc.scalar.activation(out=sqv, in_=qpart, func=AF.Square)
                    ssum = work.tile([128, 4], BF, tag="ssum")
                    with nc.allow_low_precision("rmsnorm stats tolerate bf16"):
                        nc.vector.tensor_reduce(
                            out=ssum, in_=sqv.rearrange("p (h d) -> p h d", h=4),
                            axis=mybir.AxisListType.X, op=OP.add)
                    sdev = work.tile([128, 4], F32, tag="sdev")
                    nc.scalar.activation(out=sdev, in_=ssum, func=AF.Sqrt,
                                         scale=1.0 / 64.0, bias=eps_sb)
                    rstd = work.tile([128, 4], F32, tag="rstd")
                    nc.vector.reciprocal(out=rstd, in_=sdev)
                    qs = work.tile([128, 4, 64], BF, tag="qs")
                    nc.vector.tensor_tensor(
                        out=qs, in0=qpart.rearrange("p (h d) -> p h d", h=4),
                        in1=bcast64(rstd), op=OP.mult)
                    # rope: qr = qs*cos + swap(qs)*ssin (sign folded in ssin)
                    t1 = work.tile([128, 4, 64], BF, tag="t1")
                    nc.vector.tensor_tensor(out=t1, in0=qs, in1=bcast4(cos_t),
                                            op=OP.mult)
                    t2 = work.tile([128, 4, 64], BF, tag="t2")
                    nc.vector.tensor_tensor(out=t2, in0=swap_view(qs),
                                            in1=bcast4(sin_t), op=OP.mult)
                    qr = work.tile([128, 4, 64], BF, tag="qr")
                    nc.vector.tensor_tensor(out=qr, in0=t1, in1=t2, op=OP.add)
                    pend_t.append((qr, i, dst_pair))
                    if len(pend_t) > 2:
                        flush_transpose(*pend_t.pop(0))
                    return ps

                # K/V path over all 16 chunks (qc=0 slice preloaded above)
                for qc in range(4):
                    if qc == 0:
                        c_sl = c_sl0
                    else:
                        c_sl = slices.tile([128, 8, 512], BF, tag="slice")
                        nc.sync.dma_start(out=c_sl, in_=ctxT.ap()[:, qc])
                    for ns in range(4):
                        j = qc * 4 + ns
                        ps = qk_path(c_sl, ns, j,
                                     lambda c: wkv_sb[:, c, :], 512,
                                     ck_sb[:, j, :], sk_sb[:, j, :], pairK)
                        nc.vector.tensor_copy(
                            out=v_sb[:, j, :, 0:64],
                            in_=ps[:, 256:512].rearrange("p (h d) -> p h d", h=4))

                # Q path + raw gate over all 16 chunks
                for qc in range(4):
                    x_sl = slices.tile([128, 8, 512], BF, tag="slice")
                    nc.sync.dma_start(out=x_sl, in_=xT.ap()[:, qc])
                    for ns in range(4):
                        i = qc * 4 + ns
                        qk_path(x_sl, ns, i,
                                lambda c: wq_sb[:, c, :], 256,
                                cq_sb[:, i, :], sq_sb[:, i, :], pairQ)
                    # gate projection, transposed layout, raw (sigmoid later)
                    for gfc in range(2):
                        psg = psA.tile([128, 512], F32, tag="proj")
                        for c in range(8):
                            nc.tensor.matmul(
                                psg, wg_sb[:, c, gfc * 128:(gfc + 1) * 128],
                                x_sl[:, c, :], start=(c == 0), stop=(c == 7))
                        nc.scalar.activation(
                            out=graw[:, gfc, qc * 512:(qc + 1) * 512], in_=psg,
                            func=AF.Copy)
                for e in pend_t:
                    flush_transpose(*e)
                del pend_t[:]

            # ================= P2: attention + gating + out-proj =============
            with tc.tile_pool(name="psSC", bufs=2, space="PSUM") as psSC, \
                 tc.tile_pool(name="psAO", bufs=2, space="PSUM") as psAO, \
                 tc.tile_pool(name="psDN", bufs=2, space="PSUM") as psDN:

                def emit_outproj(qc):
                    # output projection for q block qc (bf16 partial out)
                    for nk in range(4):
                        n1 = qc * 4 + nk
                        ev = gat_p.tile([128, 1024], BF, tag="ev")
                        for oc in range(2):
                            po = psDN.tile([128, 512], F32, tag="dn")
                            for fc in range(2):
                                nc.tensor.matmul(
                                    po,
                                    A_sb[:, fc, n1 * 128:(n1 + 1) * 128],
                                    wo_sb[:, fc, oc * 512:(oc + 1) * 512],
                                    start=(fc == 0), stop=(fc == 1))
                            nc.vector.tensor_copy(
                                out=ev[:, oc * 512:(oc + 1) * 512], in_=po)
                        nc.sync.dma_start(
                            out=part.ap()[n1 * 128:(n1 + 1) * 128, :], in_=ev)

                pend_op = None
                for qc in range(4):
                    qsl = slice(qc * 512, (qc + 1) * 512)
                    gat = []  # (ao_p, gs, dns) per pair, gating deferred
                    for p in range(2):
                        ao_p = psAO.tile([128, 512], F32, tag="ao")
                        dn_p = psDN.tile([128, 512], F32, tag="dn")
                        # tanh(graw/2) early: ACT slots it between exps, so
                        # the gating chain after the last attn MM is short
                        gs = gat_p.tile([128, 512], F32, tag="gs")
                        nc.scalar.activation(out=gs, in_=graw[:, p, qsl],
                                             func=AF.Tanh, scale=0.5)
                        pend = []  # (expS tile, ktc) awaiting attn MMs

                        def flush_attn(eS, k, ao_p=ao_p, dn_p=dn_p, p=p):
                            # noqa: closure over current pair
                            # first MM of the k==0 group clears the whole bank;
                            # the second must NOT re-clear (would drop the
                            # first's has_written bits) -> start only on MM1.
                            st = (k == 0)
                            sp = (k == KTC - 1)
                            nc.tensor.matmul(ao_p[0:64, :], v_sb[:, k, 2 * p, :],
                                             eS[:, 0, :], start=st, stop=sp,
                                             tile_position=(0, 0),
                                             skip_group_check=True)
                            nc.tensor.matmul(ao_p[64:128, :], v_sb[:, k, 2 * p + 1, :],
                                             eS[:, 1, :], start=st, stop=sp,
                                             tile_position=(0, 64),
                                             skip_group_check=True)
                            nc.tensor.matmul(dn_p[0:1, :], ones1, eS[:, 0, :],
                                             start=st, stop=sp,
                                             tile_position=(0, 0),
                                             skip_group_check=True)
                            nc.tensor.matmul(dn_p[32:33, :], ones1, eS[:, 1, :],
                                             start=st, stop=sp,
                                             tile_position=(0, 32),
                                             skip_group_check=True)

                        for k in range(KTC):
                            ksl = slice(k * 128, (k + 1) * 128)
                            ps = psSC.tile([128, 1024], F32, tag="sc")
                            nc.tensor.matmul(ps[:, 0:512],
                                             pairK[p][0:64, ksl],
                                             pairQ[p][0:64, qsl],
                                             start=True, stop=True,
                                             tile_position=(0, 0))
                            nc.tensor.matmul(ps[:, 512:1024],
                                             pairK[p][64:128, ksl],
                                             pairQ[p][64:128, qsl],
                                             start=True, stop=True,
                                             tile_position=(64, 0))
                            eS = exps_p.tile([128, 2, 512], BF, tag="expS")
                            if k in APPROX_KC:
                                nc.vector.tensor_scalar(
                                    out=eS.bitcast(I16).rearrange("p a b -> p (a b)"),
                                    in0=ps, scalar1=FE_A, scalar2=FE_B,
                                    op0=OP.mult, op1=OP.add)
                            else:
                                nc.scalar.activation(
                                    out=eS.rearrange("p a b -> p (a b)"), in_=ps,
                                    func=AF.Exp, scale=0.125)
                            pend.append((eS, k))
                            if len(pend) > 2:
                                flush_attn(*pend.pop(0))
                        for e in pend:
                            flush_attn(*e)

                        # dns right away (DVE only, does not block the PE);
                        # the PE part of the gating chain is deferred so it
                        # runs while the other pair's matmuls keep PE busy.
                        dns = gat_p.tile([128, 512], BF, tag="dns")
                        nc.vector.tensor_scalar_mul(out=dns, in0=dn_p,
                                                    scalar1=2.0)
                        gat.append((ao_p, gs, dns))
                        if p == 0 and pend_op is not None:
                            emit_outproj(pend_op)
                            pend_op = None

                    # gating: A = ao * sigmoid(g)/dn = ao*(tanh(g/2)+1)/(2*dn)
                    for p, (ao_p, gs, dns) in enumerate(gat):
                        rbc = psDN.tile([128, 512], F32, tag="dn")
                        nc.tensor.matmul(rbc[0:64, :], ones2[0:1, :], dns[0:1, :],
                                         start=True, stop=True,
                                         tile_position=(0, 0))
                        nc.tensor.matmul(rbc[64:128, :], ones2[32:33, :],
                                         dns[32:33, :], start=True, stop=True,
                                         tile_position=(32, 64))
                        rec = gat_p.tile([128, 512], F32, tag="rec")
                        nc.vector.reciprocal(out=rec, in_=rbc)
                        m = gat_p.tile([128, 512], F32, tag="m")
                        nc.vector.scalar_tensor_tensor(
                            out=m, in0=gs, scalar=1.0, in1=rec,
                            op0=OP.add, op1=OP.mult)
                        nc.vector.tensor_tensor(out=A_sb[:, p, qsl], in0=ao_p,
                                                in1=m, op=OP.mult)

                    pend_op = qc
                emit_outproj(pend_op)

    nc.compile()
    return nc


def _prep_core(inputs, b, g, bf16):
    x = np.asarray(inputs["x"][b], dtype=np.float32)
    ctx = np.asarray(inputs["context"][b], dtype=np.float32)
    Wq = np.asarray(inputs["Wq"], dtype=np.float32).reshape(H, 2 * D, C)
    Wkv = np.asarray(inputs["Wkv"], dtype=np.float32).reshape(H, 2 * D, C)
    Wo = np.asarray(inputs["Wo"], dtype=np.float32)
    cos = np.asarray(inputs["cos"][b], dtype=np.float32)
    sin = np.asarray(inputs["sin"][b], dtype=np.float32)
    qw = np.asarray(inputs["q_norm_w"], dtype=np.float32)
    kw = np.asarray(inputs["k_norm_w"], dtype=np.float32)

    hs = slice(HG * g, HG * g + HG)
    qr = Wq[hs, :D, :]
    qr = qr - qr.mean(axis=1, keepdims=True)
    gr = Wq[hs, D:, :]
    kr = Wkv[hs, :D, :]
    kr = kr - kr.mean(axis=1, keepdims=True)
    vr = Wkv[hs, D:, :]

    sgn = np.where(np.arange(D) < D // 2, -1.0, 1.0).astype(np.float32)
    wswap = lambda w: np.concatenate([w[D // 2:], w[:D // 2]])

    def pmajor(w, cols):
        # [cols, C] weight -> transposed, partition-major [128, 8, cols]
        return np.ascontiguousarray(
            w.reshape(cols, C).T.reshape(8, 128, cols).transpose(1, 0, 2))

    def tokmajor(t):
        # [C, N] -> [128, 4, 8, 512]: partition, q-block, c-chunk, token
        return np.ascontiguousarray(
            t.reshape(8, 128, 4, 512).transpose(1, 2, 0, 3))

    def tabs(t):
        # [N, D] -> [128, 16, D] bf16
        return np.ascontiguousarray(
            t.reshape(16, 128, D).transpose(1, 0, 2)).astype(bf16)

    return {
        "xT": tokmajor(x.T).astype(bf16),
        "ctxT": tokmajor(ctx.T).astype(bf16),
        "wq": pmajor(qr, 256).astype(bf16),
        "wg": pmajor(gr, 256).astype(bf16),
        "wkv": pmajor(
            np.concatenate([kr.reshape(HG * D, C), vr.reshape(HG * D, C)], 0),
            512).astype(bf16),
        "wo": np.ascontiguousarray(
            Wo[:, 256 * g:256 * (g + 1)].T.reshape(2, 128, C)
            .transpose(1, 0, 2)).astype(bf16),
        "cosq": tabs(cos * qw[None, :]),
        "ssinq": tabs(sin * sgn[None, :] * wswap(qw)[None, :]),
        "cosk": tabs(cos * kw[None, :]),
        "ssink": tabs(sin * sgn[None, :] * wswap(kw)[None, :]),
    }


def kernel(**inputs):
    global _PROG, LAST_EXEC_NS, LAST_PROFILE
    import ml_dtypes
    bf16 = ml_dtypes.bfloat16

    if _PROG is None:
        _PROG = _build_program()
    nc = _PROG

    in_maps = [_prep_core(inputs, core // 4, core % 4, bf16) for core in range(8)]

    trace = bool(os.environ.get("BASS_KERNEL_TRACE"))
    kw = {}
    if trace:
        import types
        from trn_agent_boot.trn_boot import _ntff_profile_via_ctypes
        hook = _ntff_profile_via_ctypes('/opt/axon/libaxon_pjrt.so')
        mod = types.ModuleType('antenv.axon_hooks')
        mod.get_axon_ntff_profile_hook = lambda: hook
        sys.modules['antenv.axon_hooks'] = mod
        from concourse import bass_utils
        bass_utils.upload_artifacts = lambda tmpdir: tmpdir
        kw = dict(trace=True, tmpdir=os.environ.get("BASS_KERNEL_TRACE_DIR"))

    from concourse.bass_utils import run_bass_kernel_spmd
    res = run_bass_kernel_spmd(nc, in_maps, core_ids=list(range(8)), **kw)
    LAST_EXEC_NS = res.exec_time_ns
    LAST_PROFILE = res.profile_json

    bo = np.asarray(inputs["bo"], dtype=np.float32)
    out = np.zeros((B, N, C), dtype=np.float32)
    for core in range(8):
        out[core // 4] += np.asarray(res.results[core]["part"],
                                     dtype=np.float32)
    out += bo[None, None, :]
    return out


# revision 12
# speedup vs baseline: 1.0994x; 1.0824x over previous
"""Trainium2 Bass kernel for nn_CrossAttention (B=2, N=2048, C=1024, H=16, D=64).

Sharding: 8 cores = 2 batches x 4 head-groups (4 heads each).
Each core computes its head-group's attention + a partial output projection;
the host sums the 4 partials per batch (bf16) and adds the bias.

Device pipeline per core:
  P1: q/k/v projections (f32r matmuls), zero-mean folded into host-centered
      weights, variance via ACT Square (bf16) + DVE reduce, rstd via
      broadcast tensor_tensor, RoPE fused on DVE in bf16 (2x/4x modes),
      bf16 PE transposes into head-paired q^T/k^T tiles, gate projected in
      transposed layout (raw, sigmoid deferred).  All input tensors are
      host-side pre-permuted so every DMA is a contiguous per-partition read;
      KV-path loads go on the sync HWDGE queue, Q-path/P2 loads on the
      scalar HWDGE queue so the first matmul starts as early as possible.
  P2: per q-block: row-tiled paired score matmuls, softmax numerators via
      ACT Exp (10/16 k-chunks, exact) or a DVE exp2 bit-trick into bf16
      (6/16 k-chunks): i16 = round(score*FE_A + FE_B) bitcast to bf16,
      col-tiled paired attn@v + M=1 ones matmuls for denominators,
      tanh-based sigmoid gating fused as (tanh+1)*1/(2*dn), bf16 output
      projection, bf16 partial out written once per 128-token row block.
"""

import os
import sys
import numpy as np

for _p in ("/opt/trn_rl_repo", "/opt/pypackages"):
    if _p not in sys.path:
        sys.path.insert(0, _p)

B, N, C = 2, 2048, 1024
H, D = 16, 64
HG = 4            # heads per core
NCH = 16          # token chunks of 128
KTC = 16          # key chunks of 128
EPS = 1e-6

# fast-exp: exp(s*0.125) ~= bitcast_bf16(int16(s*FE_A + FE_B))
FE_A = float(0.125 * np.log2(np.e) * 128.0)
FE_B = float(127.0 * 128.0 - 5.0)
APPROX_KC = (1, 3, 5, 7, 9, 11)   # DVE fast-exp chunks, early in the pair

_PROG = None      # cached compiled Bass program
LAST_EXEC_NS = None
LAST_PROFILE = None


def _build_program():
    import concourse.bass as bass
    import concourse.bacc as bacc
    import concourse.tile as tile
    import concourse.mybir as mybir

    F32 = mybir.dt.float32
    F32R = mybir.dt.float32r
    BF = mybir.dt.bfloat16
    I16 = mybir.dt.int16
    AF = mybir.ActivationFunctionType
    OP = mybir.AluOpType

    nc = bacc.Bacc("TRN2", target_bir_lowering=False, debug=False, num_devices=8)

    xT = nc.dram_tensor("xT", [128, 4, 8, 512], BF, kind="ExternalInput")
    ctxT = nc.dram_tensor("ctxT", [128, 4, 8, 512], BF, kind="ExternalInput")
    wq = nc.dram_tensor("wq", [128, 8, 256], BF, kind="ExternalInput")
    wg = nc.dram_tensor("wg", [128, 8, 256], BF, kind="ExternalInput")
    wkv = nc.dram_tensor("wkv", [128, 8, 512], BF, kind="ExternalInput")
    wo = nc.dram_tensor("wo", [128, 2, 1024], BF, kind="ExternalInput")
    cosq = nc.dram_tensor("cosq", [128, NCH, D], BF, kind="ExternalInput")
    ssinq = nc.dram_tensor("ssinq", [128, NCH, D], BF, kind="ExternalInput")
    cosk = nc.dram_tensor("cosk", [128, NCH, D], BF, kind="ExternalInput")
    ssink = nc.dram_tensor("ssink", [128, NCH, D], BF, kind="ExternalInput")
    part = nc.dram_tensor("part", [N, C], BF, kind="ExternalOutput")

    def bcast4(ap):
        # [128, 64] -> [128, 4, 64] with step-0 middle dim (read-broadcast)
        return bass.AP(tensor=ap.tensor, offset=ap.offset,
                       ap=[ap.ap[0], [0, 4], ap.ap[1]])

    def bcast64(ap):
        # [128, 4] -> [128, 4, 64] with step-0 last dim (per-head scalar)
        return bass.AP(tensor=ap.tensor, offset=ap.offset,
                       ap=[ap.ap[0], ap.ap[1], [0, 64]])

    def swap_view(ap):
        # ap: [128, 4, 64] contiguous -> per head read order d+32..d+63, d..d+31
        p, hdim, ddim = ap.ap
        return bass.AP(tensor=ap.tensor, offset=ap.offset + 32 * ddim[0],
                       ap=[p, hdim, [-32 * ddim[0], 2], [ddim[0], 32]])

    with tile.TileContext(nc) as tc:
        import contextlib
        with contextlib.ExitStack() as ctx:
            singles = ctx.enter_context(tc.tile_pool(name="singles", bufs=1))
            slices = ctx.enter_context(tc.tile_pool(name="slices", bufs=2))
            work = ctx.enter_context(tc.tile_pool(name="work", bufs=3))
            persist = ctx.enter_context(tc.tile_pool(name="persist", bufs=1))
            exps_p = ctx.enter_context(tc.tile_pool(name="exps", bufs=6))
            gat_p = ctx.enter_context(tc.tile_pool(name="gat", bufs=2))

            # ---- weights / tables.  KV-path deps go first on the sync
            # queue; Q-path and P2 loads stream in parallel on the scalar
            # HWDGE queue so the first projection matmul starts early.
            ck_sb = singles.tile([128, NCH, D], BF)
            nc.sync.dma_start(out=ck_sb, in_=cosk.ap())
            sk_sb = singles.tile([128, NCH, D], BF)
            nc.sync.dma_start(out=sk_sb, in_=ssink.ap())
            wkv_sb = singles.tile([128, 8, 512], BF)
            c_sl0 = singles.tile([128, 8, 512], BF)
            for c in range(8):
                nc.sync.dma_start(out=wkv_sb[:, c], in_=wkv.ap()[:, c])
                nc.sync.dma_start(out=c_sl0[:, c], in_=ctxT.ap()[:, 0, c])

            wq_sb = singles.tile([128, 8, 256], BF)
            nc.scalar.dma_start(out=wq_sb, in_=wq.ap())
            cq_sb = singles.tile([128, NCH, D], BF)
            nc.scalar.dma_start(out=cq_sb, in_=cosq.ap())
            sq_sb = singles.tile([128, NCH, D], BF)
            nc.scalar.dma_start(out=sq_sb, in_=ssinq.ap())
            wg_sb = singles.tile([128, 8, 256], BF)
            nc.scalar.dma_start(out=wg_sb, in_=wg.ap())
            wo_sb = singles.tile([128, 2, 1024], BF)
            nc.scalar.dma_start(out=wo_sb, in_=wo.ap())

            from concourse.masks import make_identity
            ident = singles.tile([128, 128], BF)
            make_identity(nc, ident)
            ones1 = singles.tile([128, 1], BF)
            nc.vector.memset(ones1, 1.0)
            ones2 = singles.tile([128, 64], BF)
            nc.vector.memset(ones2, 1.0)
            eps_sb = singles.tile([128, 1], F32)
            nc.vector.memset(eps_sb, EPS)

            # ---- persistent intermediates ----
            pairQ = [persist.tile([128, N], BF, tag=f"pairQ{p}", name=f"pairQ{p}") for p in range(2)]
            pairK = [persist.tile([128, N], BF, tag=f"pairK{p}", name=f"pairK{p}") for p in range(2)]
            v_sb = persist.tile([128, KTC, 4, 64], BF, tag="v_sb")
            graw = persist.tile([128, 2, N], BF, tag="graw")
            A_sb = persist.tile([128, 2, N], BF, tag="A_sb")

            # ================= P1: projections / norm / rope / transposes ====
            with tc.tile_pool(name="psA", bufs=3, space="PSUM") as psA, \
                 tc.tile_pool(name="psT", bufs=4, space="PSUM") as psT:

                pend_t = []  # (qr, i, dst_pair) transposes delayed 2 chunks

                def flush_transpose(qr, i, dst_pair):
                    # PE transpose: heads (2p, 2p+1) -> pair tile slice (bf16)
                    for p in range(2):
                        pst = psT.tile([128, 128], BF, tag="tp")
                        nc.tensor.transpose(
                            pst,
                            qr[:, 2 * p:2 * p + 2, :].rearrange("p a b -> p (a b)"),
                            ident)
                        nc.scalar.activation(
                            out=dst_pair[p][:, i * 128:(i + 1) * 128],
                            in_=pst, func=AF.Copy)

                def qk_path(sl, ns, i, w_rhs, wcols, cos_t, sin_t, dst_pair):
                    """Project+norm+rope chunk i of q or k; transpose deferred."""
                    ps = psA.tile([128, 512], F32, tag="proj")
                    for c in range(8):
                        nc.tensor.matmul(ps[:, :wcols],
                                         sl[:, c, ns * 128:(ns + 1) * 128],
                                         w_rhs(c),
                                         start=(c == 0), stop=(c == 7))
                    qpart = ps[:, 0:256]
                    # variance (zero-mean folded into host-centered weights)
                    sqv = work.tile([128, 256], BF, tag="sq")
                    nc.scalar.activation(out=sqv, in_=qpart, func=AF.Square)
                    ssum = work.tile([128, 4], BF, tag="ssum")
                    with nc.allow_low_precision("rmsnorm stats tolerate bf16"):
                        nc.vector.tensor_reduce(
                            out=ssum, in_=sqv.rearrange("p (h d) -> p h d", h=4),
                            axis=mybir.AxisListType.X, op=OP.add)
                    sdev = work.tile([128, 4], F32, tag="sdev")
                    nc.scalar.activation(out=sdev, in_=ssum, func=AF.Sqrt,
                                         scale=1.0 / 64.0, bias=eps_sb)
                    rstd = work.tile([128, 4], F32, tag="rstd")
                    nc.vector.reciprocal(out=rstd, in_=sdev)
                    qs = work.tile([128, 4, 64], BF, tag="qs")
                    nc.vector.tensor_tensor(
                        out=qs, in0=qpart.rearrange("p (h d) -> p h d", h=4),
                        in1=bcast64(rstd), op=OP.mult)
                    # rope: qr = qs*cos + swap(qs)*ssin (sign folded in ssin)
                    t1 = work.tile([128, 4, 64], BF, tag="t1")
                    nc.vector.tensor_tensor(out=t1, in0=qs, in1=bcast4(cos_t),
                                            op=OP.mult)
                    t2 = work.tile([128, 4, 64], BF, tag="t2")
                    nc.vector.tensor_tensor(out=t2, in0=swap_view(qs),
                                            in1=bcast4(sin_t), op=OP.mult)
                    qr = work.tile([128, 4, 64], BF, tag="qr")
                    nc.vector.tensor_tensor(out=qr, in0=t1, in1=t2, op=OP.add)
                    pend_t.append((qr, i, dst_pair))
                    if len(pend_t) > 2:
                        flush_transpose(*pend_t.pop(0))
                    return ps

                # K/V path over all 16 chunks (qc=0 slice preloaded above)
                for qc in range(4):
                    if qc == 0:
                        c_sl = c_sl0
                    else:
                        c_sl = slices.tile([128, 8, 512], BF, tag="slice")
                        nc.sync.dma_start(out=c_sl, in_=ctxT.ap()[:, qc])
                    for ns in range(4):
                        j = qc * 4 + ns
                        ps = qk_path(c_sl, ns, j,
                                     lambda c: wkv_sb[:, c, :], 512,
                                     ck_sb[:, j, :], sk_sb[:, j, :], pairK)
                        nc.vector.tensor_copy(
                            out=v_sb[:, j, :, 0:64],
                            in_=ps[:, 256:512].rearrange("p (h d) -> p h d", h=4))

                # Q path + raw gate over all 16 chunks
                for qc in range(4):
                    x_sl = slices.tile([128, 8, 512], BF, tag="slice")
                    nc.sync.dma_start(out=x_sl, in_=xT.ap()[:, qc])
                    for ns in range(4):
                        i = qc * 4 + ns
                        qk_path(x_sl, ns, i,
                                lambda c: wq_sb[:, c, :], 256,
                                cq_sb[:, i, :], sq_sb[:, i, :], pairQ)
                    # gate projection, transposed layout, raw (sigmoid later)
                    for gfc in range(2):
                        psg = psA.tile([128, 512], F32, tag="proj")
                        for c in range(8):
                            nc.tensor.matmul(
                                psg, wg_sb[:, c, gfc * 128:(gfc + 1) * 128],
                                x_sl[:, c, :], start=(c == 0), stop=(c == 7))
                        nc.scalar.activation(
                            out=graw[:, gfc, qc * 512:(qc + 1) * 512], in_=psg,
                            func=AF.Copy)
                for e in pend_t:
                    flush_transpose(*e)
                del pend_t[:]

            # ================= P2: attention + gating + out-proj =============
            with tc.tile_pool(name="psSC", bufs=2, space="PSUM") as psSC, \
                 tc.tile_pool(name="psAO", bufs=2, space="PSUM") as psAO, \
                 tc.tile_pool(name="psDN", bufs=2, space="PSUM") as psDN:

                def emit_outproj(qc):
                    # output projection for q block qc (bf16 partial out)
                    for nk in range(4):
                        n1 = qc * 4 + nk
                        ev = gat_p.tile([128, 1024], BF, tag="ev")
                        for oc in range(2):
                            po = psDN.tile([128, 512], F32, tag="dn")
                            for fc in range(2):
                                nc.tensor.matmul(
                                    po,
                                    A_sb[:, fc, n1 * 128:(n1 + 1) * 128],
                                    wo_sb[:, fc, oc * 512:(oc + 1) * 512],
                                    start=(fc == 0), stop=(fc == 1))
                            nc.vector.tensor_copy(
                                out=ev[:, oc * 512:(oc + 1) * 512], in_=po)
                        nc.sync.dma_start(
                            out=part.ap()[n1 * 128:(n1 + 1) * 128, :], in_=ev)

                pend_op = None
                for qc in range(4):
                    qsl = slice(qc * 512, (qc + 1) * 512)
                    gat = []  # (ao_p, gs, dns) per pair, gating deferred
                    for p in range(2):
                        ao_p = psAO.tile([128, 512], F32, tag="ao")
                        dn_p = psDN.tile([128, 512], F32, tag="dn")
                        # tanh(graw/2) early: ACT slots it between exps, so
                        # the gating chain after the last attn MM is short
                        gs = gat_p.tile([128, 512], F32, tag="gs")
                        nc.scalar.activation(out=gs, in_=graw[:, p, qsl],
                                             func=AF.Tanh, scale=0.5)
                        pend = []  # (expS tile, ktc) awaiting attn MMs

                        def flush_attn(eS, k, ao_p=ao_p, dn_p=dn_p, p=p):
                            # noqa: closure over current pair
                            # first MM of the k==0 group clears the whole bank;
                            # the second must NOT re-clear (would drop the
                            # first's has_written bits) -> start only on MM1.
                            st = (k == 0)
                            sp = (k == KTC - 1)
                            nc.tensor.matmul(ao_p[0:64, :], v_sb[:, k, 2 * p, :],
                                             eS[:, 0, :], start=st, stop=sp,
                                             tile_position=(0, 0),
                                             skip_group_check=True)
                            nc.tensor.matmul(ao_p[64:128, :], v_sb[:, k, 2 * p + 1, :],
                                             eS[:, 1, :], start=st, stop=sp,
                                             tile_position=(0, 64),
                                             skip_group_check=True)
                            nc.tensor.matmul(dn_p[0:1, :], ones1, eS[:, 0, :],
                                             start=st, stop=sp,
                                             tile_position=(0, 0),
                                             skip_group_check=True)
                            nc.tensor.matmul(dn_p[32:33, :], ones1, eS[:, 1, :],
                                             start=st, stop=sp,
                                             tile_position=(0, 32),
                                             skip_group_check=True)

                        for k in range(KTC):
                            ksl = slice(k * 128, (k + 1) * 128)
                            ps = psSC.tile([128, 1024], F32, tag="sc")
                            nc.tensor.matmul(ps[:, 0:512],
                                             pairK[p][0:64, ksl],
                                             pairQ[p][0:64, qsl],
                                             start=True, stop=True,
                                             tile_position=(0, 0))
                            nc.tensor.matmul(ps[:, 512:1024],
                                             pairK[p][64:128, ksl],
                                             pairQ[p][64:128, qsl],
                                             start=True, stop=True,
                                             tile_position=(64, 0))
                            eS = exps_p.tile([128, 2, 512], BF, tag="expS")
                            if k in APPROX_KC:
                                nc.vector.tensor_scalar(
                                    out=eS.bitcast(I16).rearrange("p a b -> p (a b)"),
                                    in0=ps, scalar1=FE_A, scalar2=FE_B,
                                    op0=OP.mult, op1=OP.add)
                            else:
                                nc.scalar.activation(
                                    out=eS.rearrange("p a b -> p (a b)"), in_=ps,
                                    func=AF.Exp, scale=0.125)
                            pend.append((eS, k))
                            if len(pend) > 3:
                                flush_attn(*pend.pop(0))
                        for e in pend:
                            flush_attn(*e)

                        # dns right away (DVE only, does not block the PE);
                        # the PE part of the gating chain is deferred so it
                        # runs while the other pair's matmuls keep PE busy.
                        dns = gat_p.tile([128, 512], BF, tag="dns")
                        nc.vector.tensor_scalar_mul(out=dns, in0=dn_p,
                                                    scalar1=2.0)
                        gat.append((ao_p, gs, dns))
                        if p == 0 and pend_op is not None:
                            emit_outproj(pend_op)
                            pend_op = None

                    # gating: A = ao * sigmoid(g)/dn = ao*(tanh(g/2)+1)/(2*dn)
                    for p, (ao_p, gs, dns) in enumerate(gat):
                        rbc = psDN.tile([128, 512], F32, tag="dn")
                        nc.tensor.matmul(rbc[0:64, :], ones2[0:1, :], dns[0:1, :],
                                         start=True, stop=True,
                                         tile_position=(0, 0))
                        nc.tensor.matmul(rbc[64:128, :], ones2[32:33, :],
                                         dns[32:33, :], start=True, stop=True,
                                         tile_position=(32, 64))
                        rec = gat_p.tile([128, 512], F32, tag="rec")
                        nc.vector.reciprocal_approx_fast(out=rec, in_=rbc)
                        m = gat_p.tile([128, 512], F32, tag="m")
                        nc.vector.scalar_tensor_tensor(
                            out=m, in0=gs, scalar=1.0, in1=rec,
                            op0=OP.add, op1=OP.mult)
                        nc.vector.tensor_tensor(out=A_sb[:, p, qsl], in0=ao_p,
                                                in1=m, op=OP.mult)

                    pend_op = qc
                emit_outproj(pend_op)

    nc.compile()
    return nc


def _prep_core(inputs, b, g, bf16):
    x = np.asarray(inputs["x"][b], dtype=np.float32)
    ctx = np.asarray(inputs["context"][b], dtype=np.float32)
    Wq = np.asarray(inputs["Wq"], dtype=np.float32).reshape(H, 2 * D, C)
    Wkv = np.asarray(inputs["Wkv"], dtype=np.float32).reshape(H, 2 * D, C)
    Wo = np.asarray(inputs["Wo"], dtype=np.float32)
    cos = np.asarray(inputs["cos"][b], dtype=np.float32)
    sin = np.asarray(inputs["sin"][b], dtype=np.float32)
    qw = np.asarray(inputs["q_norm_w"], dtype=np.float32)
    kw = np.asarray(inputs["k_norm_w"], dtype=np.float32)

    hs = slice(HG * g, HG * g + HG)
    qr = Wq[hs, :D, :]
    qr = qr - qr.mean(axis=1, keepdims=True)
    gr = Wq[hs, D:, :]
    kr = Wkv[hs, :D, :]
    kr = kr - kr.mean(axis=1, keepdims=True)
    vr = Wkv[hs, D:, :]

    sgn = np.where(np.arange(D) < D // 2, -1.0, 1.0).astype(np.float32)
    wswap = lambda w: np.concatenate([w[D // 2:], w[:D // 2]])

    def pmajor(w, cols):
        # [cols, C] weight -> transposed, partition-major [128, 8, cols]
        return np.ascontiguousarray(
            w.reshape(cols, C).T.reshape(8, 128, cols).transpose(1, 0, 2))

    def tokmajor(t):
        # [C, N] -> [128, 4, 8, 512]: partition, q-block, c-chunk, token
        return np.ascontiguousarray(
            t.reshape(8, 128, 4, 512).transpose(1, 2, 0, 3))

    def tabs(t):
        # [N, D] -> [128, 16, D] bf16
        return np.ascontiguousarray(
            t.reshape(16, 128, D).transpose(1, 0, 2)).astype(bf16)

    return {
        "xT": tokmajor(x.T).astype(bf16),
        "ctxT": tokmajor(ctx.T).astype(bf16),
        "wq": pmajor(qr, 256).astype(bf16),
        "wg": pmajor(gr, 256).astype(bf16),
        "wkv": pmajor(
            np.concatenate([kr.reshape(HG * D, C), vr.reshape(HG * D, C)], 0),
            512).astype(bf16),
        "wo": np.ascontiguousarray(
            Wo[:, 256 * g:256 * (g + 1)].T.reshape(2, 128, C)
            .transpose(1, 0, 2)).astype(bf16),
        "cosq": tabs(cos * qw[None, :]),
        "ssinq": tabs(sin * sgn[None, :] * wswap(qw)[None, :]),
        "cosk": tabs(cos * kw[None, :]),
        "ssink": tabs(sin * sgn[None, :] * wswap(kw)[None, :]),
    }


def kernel(**inputs):
    global _PROG, LAST_EXEC_NS, LAST_PROFILE
    import ml_dtypes
    bf16 = ml_dtypes.bfloat16

    if _PROG is None:
        _PROG = _build_program()
    nc = _PROG

    in_maps = [_prep_core(inputs, core // 4, core % 4, bf16) for core in range(8)]

    trace = bool(os.environ.get("BASS_KERNEL_TRACE"))
    kw = {}
    if trace:
        import types
        from trn_agent_boot.trn_boot import _ntff_profile_via_ctypes
        hook = _ntff_profile_via_ctypes('/opt/axon/libaxon_pjrt.so')
        mod = types.ModuleType('antenv.axon_hooks')
        mod.get_axon_ntff_profile_hook = lambda: hook
        sys.modules['antenv.axon_hooks'] = mod
        from concourse import bass_utils
        bass_utils.upload_artifacts = lambda tmpdir: tmpdir
        kw = dict(trace=True, tmpdir=os.environ.get("BASS_KERNEL_TRACE_DIR"))

    from concourse.bass_utils import run_bass_kernel_spmd
    res = run_bass_kernel_spmd(nc, in_maps, core_ids=list(range(8)), **kw)
    LAST_EXEC_NS = res.exec_time_ns
    LAST_PROFILE = res.profile_json

    bo = np.asarray(inputs["bo"], dtype=np.float32)
    out = np.zeros((B, N, C), dtype=np.float32)
    for core in range(8):
        out[core // 4] += np.asarray(res.results[core]["part"],
                                     dtype=np.float32)
    out += bo[None, None, :]
    return out


# revision 13
# speedup vs baseline: 1.1108x; 1.0104x over previous
"""Trainium2 Bass kernel for nn_CrossAttention (B=2, N=2048, C=1024, H=16, D=64).

Sharding: 8 cores = 2 batches x 4 head-groups (4 heads each).
Each core computes its head-group's attention + a partial output projection;
the host sums the 4 partials per batch (bf16) and adds the bias.

Device pipeline per core:
  P1: q/k/v projections (f32r matmuls), zero-mean folded into host-centered
      weights, variance via ACT Square (bf16) + DVE reduce, rstd via
      broadcast tensor_tensor, RoPE fused on DVE in bf16 (2x/4x modes),
      bf16 PE transposes into head-paired q^T/k^T tiles, gate projected in
      transposed layout (raw, sigmoid deferred).  All input tensors are
      host-side pre-permuted so every DMA is a contiguous per-partition read;
      KV-path loads go on the sync HWDGE queue, Q-path/P2 loads on the
      scalar HWDGE queue so the first matmul starts as early as possible.
  P2: per q-block: row-tiled paired score matmuls, softmax numerators via
      ACT Exp (10/16 k-chunks, exact) or a DVE exp2 bit-trick into bf16
      (6/16 k-chunks): i16 = round(score*FE_A + FE_B) bitcast to bf16,
      col-tiled paired attn@v + M=1 ones matmuls for denominators,
      tanh-based sigmoid gating fused as (tanh+1)*1/(2*dn), bf16 output
      projection, bf16 partial out written once per 128-token row block.
"""

import os
import sys
import numpy as np

for _p in ("/opt/trn_rl_repo", "/opt/pypackages"):
    if _p not in sys.path:
        sys.path.insert(0, _p)

B, N, C = 2, 2048, 1024
H, D = 16, 64
HG = 4            # heads per core
NCH = 16          # token chunks of 128
KTC = 16          # key chunks of 128
EPS = 1e-6

# fast-exp: exp(s*0.125) ~= bitcast_bf16(int16(s*FE_A + FE_B))
FE_A = float(0.125 * np.log2(np.e) * 128.0)
FE_B = float(127.0 * 128.0 - 5.0)
APPROX_KC = (1, 3, 5, 7, 9, 11)   # DVE fast-exp chunks, early in the pair

_PROG = None      # cached compiled Bass program
LAST_EXEC_NS = None
LAST_PROFILE = None


def _build_program():
    import concourse.bass as bass
    import concourse.bacc as bacc
    import concourse.tile as tile
    import concourse.mybir as mybir

    F32 = mybir.dt.float32
    F32R = mybir.dt.float32r
    BF = mybir.dt.bfloat16
    I16 = mybir.dt.int16
    AF = mybir.ActivationFunctionType
    OP = mybir.AluOpType

    nc = bacc.Bacc("TRN2", target_bir_lowering=False, debug=False, num_devices=8)

    xT = nc.dram_tensor("xT", [128, 4, 8, 512], BF, kind="ExternalInput")
    ctxT = nc.dram_tensor("ctxT", [128, 4, 8, 512], BF, kind="ExternalInput")
    wq = nc.dram_tensor("wq", [128, 8, 256], BF, kind="ExternalInput")
    wg = nc.dram_tensor("wg", [128, 8, 256], BF, kind="ExternalInput")
    wkv = nc.dram_tensor("wkv", [128, 8, 512], BF, kind="ExternalInput")
    wo = nc.dram_tensor("wo", [128, 2, 1024], BF, kind="ExternalInput")
    cosq = nc.dram_tensor("cosq", [128, NCH, D], BF, kind="ExternalInput")
    ssinq = nc.dram_tensor("ssinq", [128, NCH, D], BF, kind="ExternalInput")
    cosk = nc.dram_tensor("cosk", [128, NCH, D], BF, kind="ExternalInput")
    ssink = nc.dram_tensor("ssink", [128, NCH, D], BF, kind="ExternalInput")
    part = nc.dram_tensor("part", [N, C], BF, kind="ExternalOutput")

    def bcast4(ap):
        # [128, 64] -> [128, 4, 64] with step-0 middle dim (read-broadcast)
        return bass.AP(tensor=ap.tensor, offset=ap.offset,
                       ap=[ap.ap[0], [0, 4], ap.ap[1]])

    def bcast64(ap):
        # [128, 4] -> [128, 4, 64] with step-0 last dim (per-head scalar)
        return bass.AP(tensor=ap.tensor, offset=ap.offset,
                       ap=[ap.ap[0], ap.ap[1], [0, 64]])

    def swap_view(ap):
        # ap: [128, 4, 64] contiguous -> per head read order d+32..d+63, d..d+31
        p, hdim, ddim = ap.ap
        return bass.AP(tensor=ap.tensor, offset=ap.offset + 32 * ddim[0],
                       ap=[p, hdim, [-32 * ddim[0], 2], [ddim[0], 32]])

    with tile.TileContext(nc) as tc:
        import contextlib
        with contextlib.ExitStack() as ctx:
            singles = ctx.enter_context(tc.tile_pool(name="singles", bufs=1))
            slices = ctx.enter_context(tc.tile_pool(name="slices", bufs=3))
            work = ctx.enter_context(tc.tile_pool(name="work", bufs=5))
            persist = ctx.enter_context(tc.tile_pool(name="persist", bufs=1))
            exps_p = ctx.enter_context(tc.tile_pool(name="exps", bufs=6))
            gat_p = ctx.enter_context(tc.tile_pool(name="gat", bufs=2))

            # ---- weights / tables.  KV-path deps go first on the sync
            # queue; Q-path and P2 loads stream in parallel on the scalar
            # HWDGE queue so the first projection matmul starts early.
            ck_sb = singles.tile([128, NCH, D], BF)
            sk_sb = singles.tile([128, NCH, D], BF)
            wkv_sb = singles.tile([128, 8, 512], BF)
            c_sl0 = singles.tile([128, 8, 512], BF)
            for c in range(8):
                nc.sync.dma_start(out=wkv_sb[:, c], in_=wkv.ap()[:, c])
                nc.sync.dma_start(out=c_sl0[:, c], in_=ctxT.ap()[:, 0, c])
                if c == 1:
                    nc.sync.dma_start(out=ck_sb, in_=cosk.ap())
                    nc.sync.dma_start(out=sk_sb, in_=ssink.ap())

            wq_sb = singles.tile([128, 8, 256], BF)
            nc.scalar.dma_start(out=wq_sb, in_=wq.ap())
            cq_sb = singles.tile([128, NCH, D], BF)
            nc.scalar.dma_start(out=cq_sb, in_=cosq.ap())
            sq_sb = singles.tile([128, NCH, D], BF)
            nc.scalar.dma_start(out=sq_sb, in_=ssinq.ap())
            wg_sb = singles.tile([128, 8, 256], BF)
            nc.scalar.dma_start(out=wg_sb, in_=wg.ap())
            wo_sb = singles.tile([128, 2, 1024], BF)
            nc.scalar.dma_start(out=wo_sb, in_=wo.ap())

            from concourse.masks import make_identity
            ident = singles.tile([128, 128], BF)
            make_identity(nc, ident)
            ones1 = singles.tile([128, 1], BF)
            nc.vector.memset(ones1, 1.0)
            ones2 = singles.tile([128, 64], BF)
            nc.vector.memset(ones2, 1.0)
            eps_sb = singles.tile([128, 1], F32)
            nc.vector.memset(eps_sb, EPS)

            # ---- persistent intermediates ----
            pairQ = [persist.tile([128, N], BF, tag=f"pairQ{p}", name=f"pairQ{p}") for p in range(2)]
            pairK = [persist.tile([128, N], BF, tag=f"pairK{p}", name=f"pairK{p}") for p in range(2)]
            v_sb = persist.tile([128, KTC, 4, 64], BF, tag="v_sb")
            graw = persist.tile([128, 2, N], BF, tag="graw")
            A_sb = persist.tile([128, 2, N], BF, tag="A_sb")

            # ================= P1: projections / norm / rope / transposes ====
            with tc.tile_pool(name="psA", bufs=3, space="PSUM") as psA, \
                 tc.tile_pool(name="psT", bufs=4, space="PSUM") as psT:

                pend_t = []  # (qr, i, dst_pair) transposes delayed 2 chunks

                def flush_transpose(qr, i, dst_pair):
                    # PE transpose: heads (2p, 2p+1) -> pair tile slice (bf16)
                    for p in range(2):
                        pst = psT.tile([128, 128], BF, tag="tp")
                        nc.tensor.transpose(
                            pst,
                            qr[:, 2 * p:2 * p + 2, :].rearrange("p a b -> p (a b)"),
                            ident)
                        nc.scalar.activation(
                            out=dst_pair[p][:, i * 128:(i + 1) * 128],
                            in_=pst, func=AF.Copy)

                def qk_path(sl, ns, i, w_rhs, wcols, cos_t, sin_t, dst_pair):
                    """Project+norm+rope chunk i of q or k; transpose deferred."""
                    ps = psA.tile([128, 512], F32, tag="proj")
                    for c in range(8):
                        nc.tensor.matmul(ps[:, :wcols],
                                         sl[:, c, ns * 128:(ns + 1) * 128],
                                         w_rhs(c),
                                         start=(c == 0), stop=(c == 7))
                    qpart = ps[:, 0:256]
                    # variance (zero-mean folded into host-centered weights)
                    sqv = work.tile([128, 256], BF, tag="sq")
                    nc.scalar.activation(out=sqv, in_=qpart, func=AF.Square)
                    ssum = work.tile([128, 4], BF, tag="ssum")
                    with nc.allow_low_precision("rmsnorm stats tolerate bf16"):
                        nc.vector.tensor_reduce(
                            out=ssum, in_=sqv.rearrange("p (h d) -> p h d", h=4),
                            axis=mybir.AxisListType.X, op=OP.add)
                    sdev = work.tile([128, 4], F32, tag="sdev")
                    nc.scalar.activation(out=sdev, in_=ssum, func=AF.Sqrt,
                                         scale=1.0 / 64.0, bias=eps_sb)
                    rstd = work.tile([128, 4], F32, tag="rstd")
                    nc.vector.reciprocal(out=rstd, in_=sdev)
                    qs = work.tile([128, 4, 64], BF, tag="qs")
                    nc.vector.tensor_tensor(
                        out=qs, in0=qpart.rearrange("p (h d) -> p h d", h=4),
                        in1=bcast64(rstd), op=OP.mult)
                    # rope: qr = qs*cos + swap(qs)*ssin (sign folded in ssin)
                    t1 = work.tile([128, 4, 64], BF, tag="t1")
                    nc.vector.tensor_tensor(out=t1, in0=qs, in1=bcast4(cos_t),
                                            op=OP.mult)
                    t2 = work.tile([128, 4, 64], BF, tag="t2")
                    nc.vector.tensor_tensor(out=t2, in0=swap_view(qs),
                                            in1=bcast4(sin_t), op=OP.mult)
                    qr = work.tile([128, 4, 64], BF, tag="qr")
                    nc.vector.tensor_tensor(out=qr, in0=t1, in1=t2, op=OP.add)
                    pend_t.append((qr, i, dst_pair))
                    if len(pend_t) > 3:
                        flush_transpose(*pend_t.pop(0))
                        flush_transpose(*pend_t.pop(0))
                    return ps

                # K/V path over all 16 chunks (qc=0 slice preloaded above,
                # qc=1..3 prefetched on the sync queue before compute starts)
                csls = [c_sl0]
                for qc in range(1, 4):
                    c_sl = slices.tile([128, 8, 512], BF, tag="slice")
                    nc.sync.dma_start(out=c_sl, in_=ctxT.ap()[:, qc])
                    csls.append(c_sl)
                for qc in range(4):
                    c_sl = csls[qc]
                    for ns in range(4):
                        j = qc * 4 + ns
                        ps = qk_path(c_sl, ns, j,
                                     lambda c: wkv_sb[:, c, :], 512,
                                     ck_sb[:, j, :], sk_sb[:, j, :], pairK)
                        nc.vector.tensor_copy(
                            out=v_sb[:, j, :, 0:64],
                            in_=ps[:, 256:512].rearrange("p (h d) -> p h d", h=4))

                # Q path + raw gate over all 16 chunks
                for qc in range(4):
                    x_sl = slices.tile([128, 8, 512], BF, tag="slice")
                    nc.sync.dma_start(out=x_sl, in_=xT.ap()[:, qc])
                    for ns in range(4):
                        i = qc * 4 + ns
                        qk_path(x_sl, ns, i,
                                lambda c: wq_sb[:, c, :], 256,
                                cq_sb[:, i, :], sq_sb[:, i, :], pairQ)
                    # gate projection, transposed layout, raw (sigmoid later)
                    for gfc in range(2):
                        psg = psA.tile([128, 512], F32, tag="proj")
                        for c in range(8):
                            nc.tensor.matmul(
                                psg, wg_sb[:, c, gfc * 128:(gfc + 1) * 128],
                                x_sl[:, c, :], start=(c == 0), stop=(c == 7))
                        nc.scalar.activation(
                            out=graw[:, gfc, qc * 512:(qc + 1) * 512], in_=psg,
                            func=AF.Copy)
                for e in pend_t:
                    flush_transpose(*e)
                del pend_t[:]

            # ================= P2: attention + gating + out-proj =============
            with tc.tile_pool(name="psSC", bufs=2, space="PSUM") as psSC, \
                 tc.tile_pool(name="psAO", bufs=2, space="PSUM") as psAO, \
                 tc.tile_pool(name="psDN", bufs=2, space="PSUM") as psDN:

                def emit_outproj(qc):
                    # output projection for q block qc (bf16 partial out)
                    for nk in range(4):
                        n1 = qc * 4 + nk
                        ev = gat_p.tile([128, 1024], BF, tag="ev")
                        for oc in range(2):
                            po = psDN.tile([128, 512], F32, tag="dn")
                            for fc in range(2):
                                nc.tensor.matmul(
                                    po,
                                    A_sb[:, fc, n1 * 128:(n1 + 1) * 128],
                                    wo_sb[:, fc, oc * 512:(oc + 1) * 512],
                                    start=(fc == 0), stop=(fc == 1))
                            nc.vector.tensor_copy(
                                out=ev[:, oc * 512:(oc + 1) * 512], in_=po)
                        nc.sync.dma_start(
                            out=part.ap()[n1 * 128:(n1 + 1) * 128, :], in_=ev)

                pend_op = None
                for qc in range(4):
                    qsl = slice(qc * 512, (qc + 1) * 512)
                    gat = []  # (ao_p, gs, dns) per pair, gating deferred
                    for p in range(2):
                        ao_p = psAO.tile([128, 512], F32, tag="ao")
                        dn_p = psDN.tile([128, 512], F32, tag="dn")
                        # tanh(graw/2) early: ACT slots it between exps, so
                        # the gating chain after the last attn MM is short
                        gs = gat_p.tile([128, 512], F32, tag="gs")
                        nc.scalar.activation(out=gs, in_=graw[:, p, qsl],
                                             func=AF.Tanh, scale=0.5)
                        pend = []  # (expS tile, ktc) awaiting attn MMs

                        def flush_attn(eS, k, ao_p=ao_p, dn_p=dn_p, p=p):
                            # noqa: closure over current pair
                            # first MM of the k==0 group clears the whole bank;
                            # the second must NOT re-clear (would drop the
                            # first's has_written bits) -> start only on MM1.
                            st = (k == 0)
                            sp = (k == KTC - 1)
                            nc.tensor.matmul(ao_p[0:64, :], v_sb[:, k, 2 * p, :],
                                             eS[:, 0, :], start=st, stop=sp,
                                             tile_position=(0, 0),
                                             skip_group_check=True)
                            nc.tensor.matmul(ao_p[64:128, :], v_sb[:, k, 2 * p + 1, :],
                                             eS[:, 1, :], start=st, stop=sp,
                                             tile_position=(0, 64),
                                             skip_group_check=True)
                            nc.tensor.matmul(dn_p[0:1, :], ones1, eS[:, 0, :],
                                             start=st, stop=sp,
                                             tile_position=(0, 0),
                                             skip_group_check=True)
                            nc.tensor.matmul(dn_p[32:33, :], ones1, eS[:, 1, :],
                                             start=st, stop=sp,
                                             tile_position=(0, 32),
                                             skip_group_check=True)

                        for k in range(KTC):
                            ksl = slice(k * 128, (k + 1) * 128)
                            ps = psSC.tile([128, 1024], F32, tag="sc")
                            nc.tensor.matmul(ps[:, 0:512],
                                             pairK[p][0:64, ksl],
                                             pairQ[p][0:64, qsl],
                                             start=True, stop=True,
                                             tile_position=(0, 0))
                            nc.tensor.matmul(ps[:, 512:1024],
                                             pairK[p][64:128, ksl],
                                             pairQ[p][64:128, qsl],
                                             start=True, stop=True,
                                             tile_position=(64, 0))
                            eS = exps_p.tile([128, 2, 512], BF, tag="expS")
                            if k in APPROX_KC:
                                nc.vector.tensor_scalar(
                                    out=eS.bitcast(I16).rearrange("p a b -> p (a b)"),
                                    in0=ps, scalar1=FE_A, scalar2=FE_B,
                                    op0=OP.mult, op1=OP.add)
                            else:
                                nc.scalar.activation(
                                    out=eS.rearrange("p a b -> p (a b)"), in_=ps,
                                    func=AF.Exp, scale=0.125)
                            pend.append((eS, k))
                            if len(pend) > 3:
                                flush_attn(*pend.pop(0))
                        for e in pend:
                            flush_attn(*e)

                        # dns right away (DVE only, does not block the PE);
                        # the PE part of the gating chain is deferred so it
                        # runs while the other pair's matmuls keep PE busy.
                        dns = gat_p.tile([128, 512], BF, tag="dns")
                        nc.vector.tensor_scalar_mul(out=dns, in0=dn_p,
                                                    scalar1=2.0)
                        gat.append((ao_p, gs, dns))
                        if p == 0 and pend_op is not None:
                            emit_outproj(pend_op)
                            pend_op = None

                    # gating: A = ao * sigmoid(g)/dn = ao*(tanh(g/2)+1)/(2*dn)
                    for p, (ao_p, gs, dns) in enumerate(gat):
                        rbc = psDN.tile([128, 512], F32, tag="dn")
                        nc.tensor.matmul(rbc[0:64, :], ones2[0:1, :], dns[0:1, :],
                                         start=True, stop=True,
                                         tile_position=(0, 0))
                        nc.tensor.matmul(rbc[64:128, :], ones2[32:33, :],
                                         dns[32:33, :], start=True, stop=True,
                                         tile_position=(32, 64))
                        rec = gat_p.tile([128, 512], F32, tag="rec")
                        nc.vector.reciprocal_approx_fast(out=rec, in_=rbc)
                        m = gat_p.tile([128, 512], F32, tag="m")
                        nc.vector.scalar_tensor_tensor(
                            out=m, in0=gs, scalar=1.0, in1=rec,
                            op0=OP.add, op1=OP.mult)
                        nc.vector.tensor_tensor(out=A_sb[:, p, qsl], in0=ao_p,
                                                in1=m, op=OP.mult)

                    pend_op = qc
                emit_outproj(pend_op)

    nc.compile()
    return nc


def _prep_core(inputs, b, g, bf16):
    x = np.asarray(inputs["x"][b], dtype=np.float32)
    ctx = np.asarray(inputs["context"][b], dtype=np.float32)
    Wq = np.asarray(inputs["Wq"], dtype=np.float32).reshape(H, 2 * D, C)
    Wkv = np.asarray(inputs["Wkv"], dtype=np.float32).reshape(H, 2 * D, C)
    Wo = np.asarray(inputs["Wo"], dtype=np.float32)
    cos = np.asarray(inputs["cos"][b], dtype=np.float32)
    sin = np.asarray(inputs["sin"][b], dtype=np.float32)
    qw = np.asarray(inputs["q_norm_w"], dtype=np.float32)
    kw = np.asarray(inputs["k_norm_w"], dtype=np.float32)

    hs = slice(HG * g, HG * g + HG)
    qr = Wq[hs, :D, :]
    qr = qr - qr.mean(axis=1, keepdims=True)
    gr = Wq[hs, D:, :]
    kr = Wkv[hs, :D, :]
    kr = kr - kr.mean(axis=1, keepdims=True)
    vr = Wkv[hs, D:, :]

    sgn = np.where(np.arange(D) < D // 2, -1.0, 1.0).astype(np.float32)
    wswap = lambda w: np.concatenate([w[D // 2:], w[:D // 2]])

    def pmajor(w, cols):
        # [cols, C] weight -> transposed, partition-major [128, 8, cols]
        return np.ascontiguousarray(
            w.reshape(cols, C).T.reshape(8, 128, cols).transpose(1, 0, 2))

    def tokmajor(t):
        # [C, N] -> [128, 4, 8, 512]: partition, q-block, c-chunk, token
        return np.ascontiguousarray(
            t.reshape(8, 128, 4, 512).transpose(1, 2, 0, 3))

    def tabs(t):
        # [N, D] -> [128, 16, D] bf16
        return np.ascontiguousarray(
            t.reshape(16, 128, D).transpose(1, 0, 2)).astype(bf16)

    return {
        "xT": tokmajor(x.T).astype(bf16),
        "ctxT": tokmajor(ctx.T).astype(bf16),
        "wq": pmajor(qr, 256).astype(bf16),
        "wg": pmajor(gr, 256).astype(bf16),
        "wkv": pmajor(
            np.concatenate([kr.reshape(HG * D, C), vr.reshape(HG * D, C)], 0),
            512).astype(bf16),
        "wo": np.ascontiguousarray(
            Wo[:, 256 * g:256 * (g + 1)].T.reshape(2, 128, C)
            .transpose(1, 0, 2)).astype(bf16),
        "cosq": tabs(cos * qw[None, :]),
        "ssinq": tabs(sin * sgn[None, :] * wswap(qw)[None, :]),
        "cosk": tabs(cos * kw[None, :]),
        "ssink": tabs(sin * sgn[None, :] * wswap(kw)[None, :]),
    }


def kernel(**inputs):
    global _PROG, LAST_EXEC_NS, LAST_PROFILE
    import ml_dtypes
    bf16 = ml_dtypes.bfloat16

    if _PROG is None:
        _PROG = _build_program()
    nc = _PROG

    in_maps = [_prep_core(inputs, core // 4, core % 4, bf16) for core in range(8)]

    trace = bool(os.environ.get("BASS_KERNEL_TRACE"))
    kw = {}
    if trace:
        import types
        from trn_agent_boot.trn_boot import _ntff_profile_via_ctypes
        hook = _ntff_profile_via_ctypes('/opt/axon/libaxon_pjrt.so')
        mod = types.ModuleType('antenv.axon_hooks')
        mod.get_axon_ntff_profile_hook = lambda: hook
        sys.modules['antenv.axon_hooks'] = mod
        from concourse import bass_utils
        bass_utils.upload_artifacts = lambda tmpdir: tmpdir
        kw = dict(trace=True, tmpdir=os.environ.get("BASS_KERNEL_TRACE_DIR"))

    from concourse.bass_utils import run_bass_kernel_spmd
    res = run_bass_kernel_spmd(nc, in_maps, core_ids=list(range(8)), **kw)
    LAST_EXEC_NS = res.exec_time_ns
    LAST_PROFILE = res.profile_json

    bo = np.asarray(inputs["bo"], dtype=np.float32)
    out = np.zeros((B, N, C), dtype=np.float32)
    for core in range(8):
        out[core // 4] += np.asarray(res.results[core]["part"],
                                     dtype=np.float32)
    out += bo[None, None, :]
    return out
